# revision 34
# baseline (speedup 1.0000x reference)
"""Trainium2 Bass kernel: greedy bbox-matching loss (nn_BboxLoss).

Sharding: 4 pred-shards x 2 target-halves over 8 NeuronCores. Within each
shard, preds are HOST-PERMUTED into area-sorted order so that slot s's 32
mates (area-ranks [32s, 32s+32)) are CONTIGUOUS. Per core, per row-tile j of
[128 targets x 2048 preds]:

  pwx = relu(min(x1p, x1t) - max(x0p, x0t))   [custom DVE op, fp16 in,
  pwy = relu(min(y1p, y1t) - max(y0p, y0t))    bf16 out, 2x perf mode]
  m3  = max over each 32-mate slot of pwx*pwy [custom DVE subdim op, 2x]
  m3 page-max columns compacted on GPSIMD and DMA'd out.

Both custom DVE ops carry hand-authored 2X_1PORT uOp programs (element 1 via
SRC_*_HI, dual WR0_LO/WR0_HI writes — validated against numpy references by
a uop-pipeline simulator) and set perf_max=1 so the engine/cost model run
them at 2 elem/cycle.

The host computes kq = m3 * RSU in f32 (RSU[t, s] = 1/(min-mate-area + at
+ eps)) and takes the top-8 slots per shard. kq[s] upper-bounds
key = inter/S of every mate in slot s (m3 >= inter_p, RSU >= RS_p), and
because mates have adjacent areas it is also a tight estimate, so top-8
slots by kq track the top preds by IoU. Host expands each top-8 slot to
its 32 mates (4 shards x 8 slots x 32 = 1024 candidates per row),
evaluates exact reference losses, and runs the greedy walk.
Each pick is certified sound: non-candidate preds have device-key <= v8
(the 8th slot-max), so their exact loss is floor-bounded via
u_cap = v8*(1+eps_round) + fp16-plane slack; picks that cannot be
certified fall back to an exact full-row argmin. The final loss is the
exact reference-form loss of the selected pairs.
"""
import numpy as np
from contextlib import ExitStack

P_TOTAL = 8192
T = 2048
N_CORES = 8
NP_SHARD = 4
NT_SHARD = 2
P_CORE = P_TOTAL // NP_SHARD   # 2048
T_CORE = T // NT_SHARD         # 1024
NJ = T_CORE // 128             # 8
EPS = 1e-7
NSLOT = 64
AMB = P_CORE // NSLOT          # 32 mates per slot

_CACHE = {}
_PM_PW = 1      # perf_max for the PW op (debug knob)
_PM_IMAX = 1    # perf_max for the IMAX op (debug knob)


# --------------------------------------------------------------------------
# custom DVE ops: PW (with 2x variant) and IMAX (mult + max-over-subdim)
# --------------------------------------------------------------------------
def _pw_2x_uops():
    """relu(min(Src0,C0) - max(Src1,C1)) for the packed (lo, hi) pair.
    lanes: d0=SRC_1, d1=C0, d2=C1, d3=ZERO, d4=SRC_0_HI, d5=SRC_1_HI."""
    from concourse.dve_uop import (
        ENABLE, AluInp, AluOp, DelayInp, InpSel, OutPath, OutSel, Trigger,
        UopConfig, UopDpConfig,
    )

    PREV = AluInp.PREV_ALU_OUT
    D = [AluInp.PREV_DELAY_0 + i for i in range(6)]

    def dp8(nl):
        blocks = []
        for _ in range(8):
            b = UopDpConfig()
            b.pass_through_delay(*range(nl))
            blocks.append(b)
        return blocks

    def cap(b, lane):
        b.delay[lane] = DelayInp.PREV_ALU_OUT
        b.delay_enable[lane] = ENABLE
        return b

    u = UopConfig()
    u.enable_input(InpSel.SRC_0, 0)
    u.enable_input(InpSel.SRC_1, 1)
    u.enable_input(InpSel.CONST_0, 2)
    u.enable_input(InpSel.CONST_1, 3)
    u.enable_input(InpSel.ZERO, 4)
    u.enable_input(InpSel.SRC_0_HI, 5)
    u.enable_input(InpSel.SRC_1_HI, 6)
    dp = dp8(6)
    dp[0].enable_alu(AluOp.MIN, PREV, D[1])           # min_lo
    cap(dp[1].enable_alu(AluOp.MAX, D[0], D[2]), 0)   # max_lo; d0 <- min_lo
    dp[2].enable_alu(AluOp.SUBTRACT, D[0], PREV)      # diff_lo
    dp[3].enable_alu(AluOp.MAX, PREV, D[3])           # pw_lo = relu
    cap(dp[4].enable_alu(AluOp.MIN, D[4], D[1]), 4)   # min_hi; d4 <- pw_lo
    cap(dp[5].enable_alu(AluOp.MAX, D[5], D[2]), 5)   # max_hi; d5 <- min_hi
    dp[6].enable_alu(AluOp.SUBTRACT, D[5], PREV)      # diff_hi
    dp[7].enable_alu(AluOp.MAX, PREV, D[3])           # pw_hi = relu
    u.datapath_config = dp
    u.enable_output(OutSel.DELAY_4, OutPath.WR0_LO)
    u.enable_output(OutSel.ALU_OUT, OutPath.WR0_HI)
    u.require_inp0 = u.require_inp1 = 1
    u.trigger = (Trigger.SRC_TENSOR_DONE, Trigger.NONE, Trigger.NONE)
    u.next_uop = (0, 0, 0)
    return [u]


def _imax_uops(pair):
    """Full-rate running max of (Src0*Src1) with reset at each N-element
    page (SUB_DIM_DONE); the page max lands in the page's LAST output
    element. 3 states: entry-reset, steady, mid-reset (the PageIdx
    step-state idiom). Full-rate writes because write_subdim_last
    misbehaves in 2X_1PORT mode (HW-verified)."""
    from concourse.dve_uop import (
        ENABLE, AluInp, AluOp, DelayInp, InpSel, OutPath, OutSel, Trigger,
        UopConfig, UopDpConfig,
    )

    PREV = AluInp.PREV_ALU_OUT
    CURR = AluInp.CURR_ALU_OUT
    D = [AluInp.PREV_DELAY_0 + i for i in range(6)]
    SRC_DONE, SUBD, CNT = (
        Trigger.SRC_TENSOR_DONE, Trigger.SUB_DIM_DONE, Trigger.COUNT,
    )

    def build(reset):
        u = UopConfig()
        u.enable_input(InpSel.SRC_0, 0)
        u.enable_input(InpSel.SRC_1, 1)          # d0
        nl = 1
        if pair == 2:
            u.enable_input(InpSel.SRC_0_HI, 2)   # d1
            u.enable_input(InpSel.SRC_1_HI, 3)   # d2
            nl = 3
        dp = []
        for _ in range(8):
            b = UopDpConfig()
            b.pass_through_delay(*range(nl))
            dp.append(b)
        if pair == 1:
            dp[0].enable_alu(AluOp.MULTIPLY, PREV, D[0])
            if reset:
                dp[1].enable_alu(AluOp.BYPASS, PREV, PREV)
            else:
                dp[1].enable_alu(AluOp.MAX, CURR, PREV)
            tail = 2
        else:
            dp[0].enable_alu(AluOp.MULTIPLY, PREV, D[0])       # v_lo
            b1 = dp[1].enable_alu(AluOp.MULTIPLY, D[1], D[2])  # v_hi
            b1.delay[0] = DelayInp.PREV_ALU_OUT                # d0 <- v_lo
            b1.delay_enable[0] = ENABLE
            dp[2].enable_alu(AluOp.MAX, D[0], PREV)            # pair max
            if reset:
                dp[3].enable_alu(AluOp.BYPASS, PREV, PREV)
            else:
                dp[3].enable_alu(AluOp.MAX, CURR, PREV)
            tail = 4
        for st in range(tail, 8):
            dp[st].enable_alu(AluOp.BYPASS, PREV, PREV)
        u.datapath_config = dp
        u.enable_output(OutSel.ALU_OUT, OutPath.WR0_LO)
        if pair == 2:
            u.enable_output(OutSel.ALU_OUT, OutPath.WR0_HI)
        u.require_inp0 = u.require_inp1 = 1
        return u

    us = []
    for reset, trig, nxt, rpt in (
        (True, (SRC_DONE, SUBD, CNT), (0, 2, 1), 1),
        (False, (SRC_DONE, SUBD, Trigger.NONE), (0, 2, 0), 0),
        (True, (SRC_DONE, SUBD, CNT), (0, 2, 1), 1),
    ):
        u = build(reset)
        u.trigger = trig
        u.next_uop = nxt
        u.repeat_count = rpt
        us.append(u)
    return us


def _get_dve_ops():
    """Register PW_BBOX_ANT (with 2x variant) and IMAX_SLOT_ANT (1x + 2x,
    subdim reduce). Returns (pw_op, imax_op). The compiled DveOpSpecs are
    seeded into dve_ops._COMPILE_CACHE so dve_table_for_ops writes the
    perf-mode table slots."""
    from concourse.dve_spec import Spec, Src0, Src1, C0, C1, relu, maxx, minn, lower
    from concourse import dve_ops
    from concourse.dve_uop import DveOpSpec

    pw_name, im_name = "PW_BBOX_ANT", "IMAX_SLOT_ANT"
    if pw_name in dve_ops._SUB_OPCODE_FOR_NAME:
        by_name = {o.name: o for o in dve_ops.OPS}
        return by_name[pw_name], by_name[im_name]

    pw_spec = Spec(
        body=relu(minn(Src0, C0) - maxx(Src1, C1)),
        reference=lambda in0, in1, s0, s1, imm2: np.maximum(
            np.minimum(in0.astype(np.float32), s0)
            - np.maximum(in1.astype(np.float32), s1),
            0.0,
        ).astype(np.float32),
    )

    def im_ref(in0, in1, s0, s1, imm2):
        v = in0.astype(np.float32) * in1.astype(np.float32)
        v = v.reshape(v.shape[0], -1, AMB)
        return np.maximum.accumulate(v, axis=-1).reshape(in0.shape)

    im_spec = Spec(body=Src0 * Src1, reference=im_ref)

    row0 = max(dve_ops._SUB_OPCODE_FOR_NAME.values()) + 1
    assert row0 + 1 < 0x20
    out_ops = []
    for name, spec, row, uops_1x, uops_2x, subdim in (
        (pw_name, pw_spec, row0, lower(pw_spec, ver="v3"), _pw_2x_uops(), False),
        (im_name, im_spec, row0 + 1, _imax_uops(1), _imax_uops(2), True),
    ):
        dve_ops._SUB_OPCODE_FOR_NAME[name] = row
        compiled = DveOpSpec(
            name=name, opcode=row, uops=uops_1x, uops_2x=uops_2x,
            rd1_en=True, perf_max=1,
        )
        compiled.validate("v3")
        op = dve_ops.DveOp(name, spec, subdim=subdim, uops_sha={})
        dve_ops.OPS.append(op)
        dve_ops.CUSTOM_DVE_SPECS[name] = spec
        dve_ops._COMPILE_CACHE[(name, "v3")] = compiled
        out_ops.append(op)
    return out_ops[0], out_ops[1]


# --------------------------------------------------------------------------
# device program
# --------------------------------------------------------------------------
def _build_nc():
    import concourse.bacc as bacc
    import concourse.mybir as mybir
    from concourse.tile import TileContext

    f32 = mybir.dt.float32
    bf16 = mybir.dt.bfloat16
    fp16 = mybir.dt.float16
    Alu = mybir.AluOpType
    pw_op, imax_op = _get_dve_ops()

    nc = bacc.Bacc()
    pl_d = nc.dram_tensor("planes", [128, 4 * P_CORE], fp16, kind="ExternalInput")
    tsc_d = nc.dram_tensor("tscal", [128, 6 * NJ], f32, kind="ExternalInput")
    out_d = nc.dram_tensor("m3c", [128, NJ * NSLOT], bf16, kind="ExternalOutput")

    Act = mybir.ActivationFunctionType
    with TileContext(nc) as tc, ExitStack() as ctx:
        const = ctx.enter_context(tc.tile_pool(name="const", bufs=1))
        work = ctx.enter_context(tc.tile_pool(name="work", bufs=4))
        red = ctx.enter_context(tc.tile_pool(name="red", bufs=4))
        actw = ctx.enter_context(tc.tile_pool(name="actw", bufs=2))

        TSC = const.tile([128, 6, NJ], f32)
        XPQ = const.tile([128, 2, P_CORE], fp16)
        YPQ = const.tile([128, 2, P_CORE], fp16)
        X1P, X0P = XPQ[:, 0, :], XPQ[:, 1, :]
        Y1P, Y0P = YPQ[:, 0, :], YPQ[:, 1, :]
        M3C = const.tile([128, NJ, NSLOT], bf16)

        H = P_CORE // 2
        C = 256
        # dram layout: [x1p_c|x0p_c|x1p_r|x0p_r|y1p_h0|y0p_h0|y1p_h1|y0p_h1]
        # (c = first 256 cols, r = rest of h0..h1) — tiny first chunk so the
        # DVE starts ~3.3us in
        # y planes issued from the idle PE sequencer: a DMA's sem-wait holds
        # its issuer's SEQ, so keeping queues separate avoids serialization
        nc.sync.dma_start(XPQ[:, :, :C], pl_d[:, 0 : 2 * C])
        nc.sync.dma_start(TSC[:].rearrange("p q j -> p (q j)"), tsc_d[:])
        nc.scalar.dma_start(YPQ[:, :, :H], pl_d[:, 4 * H : 6 * H])
        nc.sync.dma_start(XPQ[:, :, C:H], pl_d[:, 2 * C : 2 * H])
        nc.scalar.dma_start(YPQ[:, :, H:], pl_d[:, 6 * H : 8 * H])
        nc.sync.dma_start(XPQ[:, :, H:], pl_d[:, 2 * H : 4 * H])

        PWX = [None] * NJ
        PWY = [None] * NJ
        M3 = [None] * NJ

        def pw(j, d, slices=(slice(None),)):
            hi = TSC[:, 0 if d == "x" else 2, j : j + 1]
            lo = TSC[:, 1 if d == "x" else 3, j : j + 1]
            P1, P0 = (X1P, X0P) if d == "x" else (Y1P, Y0P)
            if d == "x" and PWX[j] is None or d == "y" and PWY[j] is None:
                dst = work.tile(
                    [128, NSLOT, AMB], bf16, tag=f"pw{d}", name=f"pw{d}{j}"
                )
                if d == "x":
                    PWX[j] = dst
                else:
                    PWY[j] = dst
            dst = PWX[j] if d == "x" else PWY[j]
            flat = dst[:].rearrange("p s n -> p (s n)")
            for sl in slices:
                nc.vector._custom_dve(
                    pw_op, out=flat[:, sl], in0=P1[:, sl], in1=P0[:, sl],
                    s0=hi, s1=lo,
                ).ins.perf_max = _PM_PW

        def imax(j, half=None):
            if M3[j] is None:
                M3[j] = red.tile(
                    [128, NSLOT, AMB], bf16, tag="m3", name=f"m3_{j}"
                )
            hs = slice(None) if half is None else (
                slice(0, NSLOT // 2) if half == 0 else slice(NSLOT // 2, NSLOT)
            )
            nc.vector._custom_dve(
                imax_op, out=M3[j][:, hs], in0=PWX[j][:, hs], in1=PWY[j][:, hs],
                s0=0.0, s1=0.0,
            ).ins.perf_max = _PM_IMAX

        def compact(j):
            # page max sits in each page's last element; gather to [128,64].
            # On DVE (127ns): Pool's in-order queue is owned by the offload
            # adds and would gate every output DMA behind them.
            nc.vector.tensor_scalar(
                M3C[:, j, :], M3[j][:, :, AMB - 1], 1.0, None, op0=Alu.mult
            )
            # late tiles' DMAs from PE: their sem-waits must not block the
            # SP queue that carries the early tiles' (already-ready) DMAs
            eng = nc.scalar if j >= 4 else nc.sync
            eng.dma_start(out_d[:, j * NSLOT : (j + 1) * NSLOT], M3C[:, j, :])

        def compact2(j, half):
            hs = slice(half * NSLOT // 2, (half + 1) * NSLOT // 2)
            nc.vector.tensor_scalar(
                M3C[:, j, hs], M3[j][:, hs, AMB - 1], 1.0, None, op0=Alu.mult
            )
            lo = j * NSLOT + half * NSLOT // 2
            nc.scalar.dma_start(out_d[:, lo : lo + NSLOT // 2], M3C[:, j, hs])

        # pwy for the last OFF tiles runs on ACT+Pool instead of the DVE:
        # pwy = relu(sy - P - Q), P = relu(y1t - y1p), Q = relu(y0p - y0t),
        # sy = y1t - y0t. f32 intermediates: single bf16 rounding at the end,
        # same error budget as the DVE pw path.
        OFF = (5, 6, 7)

        def act_front(j):
            P = actw.tile([128, P_CORE], f32, tag="actp", name=f"actp{j}")
            Q = actw.tile([128, P_CORE], f32, tag="actq", name=f"actq{j}")
            nc.scalar.activation(
                P[:], Y1P[:], Act.Relu, bias=TSC[:, 2, j : j + 1], scale=-1.0
            )
            nc.scalar.activation(
                Q[:], Y0P[:], Act.Relu, bias=TSC[:, 4, j : j + 1], scale=1.0
            )
            return P, Q

        def act_add(j, P, Q):
            Tt = actw.tile([128, P_CORE], f32, tag="actt", name=f"actt{j}")
            nc.gpsimd.tensor_tensor(Tt[:], P[:], Q[:], op=Alu.add)
            return Tt

        def act_back(j, Tt):
            PWY[j] = work.tile(
                [128, NSLOT, AMB], bf16, tag="pwy", name=f"pwy{j}"
            )
            nc.scalar.activation(
                PWY[j][:].rearrange("p s n -> p (s n)"), Tt[:], Act.Relu,
                bias=TSC[:, 5, j : j + 1], scale=-1.0,
            )

        # tiles 0-1 fully split into h0/h1 (pw AND imax halves) so phase-A
        # work needs only the first half-planes — the DVE never waits for
        # x_h1/y_h1
        lo, hi = slice(0, H), slice(H, P_CORE)
        pw(0, "x", (slice(0, C), slice(C, H)))
        pw(0, "y", (lo,))
        imax(0, half=0)
        pw(1, "x", (lo,))
        pw(1, "y", (lo,))
        imax(1, half=0)
        pw(0, "x", (hi,))
        pw(0, "y", (hi,))
        imax(0, half=1)
        pw(1, "x", (hi,))
        pw(1, "y", (hi,))
        imax(1, half=1)
        # ACT front passes for the offloaded tiles, then the Pool adds; the
        # final relus are emitted in order behind them on the ACT queue.
        # high_priority: the scheduler must NOT queue these behind compacts
        # (the pwy chain has a 3-hop latency the late tiles depend on).
        with tc.high_priority():
            PQ = {j: act_front(j) for j in OFF}
            TT = {j: act_add(j, *PQ[j]) for j in OFF}
            for j in OFF:
                act_back(j, TT[j])
        # 2-deep software pipeline: compact trails imax by one tile
        for j in range(2, NJ):
            pw(j, "x")
            if j not in OFF:
                pw(j, "y")
            if j > 2:
                imax(j - 1)
            compact(j - 2)
        # tail: last tile's imax/compact/DMA split in halves so the first
        # half's DMA overlaps the second half's compute
        imax(NJ - 1, half=0)
        compact2(NJ - 1, 0)
        imax(NJ - 1, half=1)
        compact(NJ - 2)
        compact2(NJ - 1, 1)

    # 2x perf mode for the custom ops (uops_2x present in the table;
    # engine falls back to 1x if the mem-pattern does not qualify)
    for b in nc.m.functions[0].blocks:
        for inst in b.instructions:
            if type(inst).__name__ == "InstCustomDveAnt":
                inst.perf_max = (
                    _PM_PW if inst.op_name == "PW_BBOX_ANT" else _PM_IMAX
                )

    nc.compile()
    return nc


# --------------------------------------------------------------------------
# host side
# --------------------------------------------------------------------------
def _clip_planes(pred):
    x1p = np.minimum(pred[:, 0] + pred[:, 2] / 2, np.float32(1.0))
    x0p = np.maximum(pred[:, 0] - pred[:, 2] / 2, np.float32(0.0))
    y1p = np.minimum(pred[:, 1] + pred[:, 3] / 2, np.float32(1.0))
    y0p = np.maximum(pred[:, 1] - pred[:, 3] / 2, np.float32(0.0))
    return x1p, x0p, y1p, y0p


def _shard_perm(ap_shard):
    """Area-sort permutation: position r holds area-rank r, so slot s's
    mates are the contiguous area-ranks [s*AMB, (s+1)*AMB)."""
    return np.argsort(ap_shard, kind="stable")


def _prep_core_inputs(pred, tgt):
    """Build per-core input dicts. pred [P,4], tgt [T,4] float32.
    Returns (in_maps, perms, rsus) with perms[px][new_pos] = local orig idx
    and rsus[px] = f32 [T, NSLOT] = 1/(min-mate-area + at + eps)."""
    x1t = tgt[:, 0] + tgt[:, 2] / 2
    x0t = tgt[:, 0] - tgt[:, 2] / 2
    y1t = tgt[:, 1] + tgt[:, 3] / 2
    y0t = tgt[:, 1] - tgt[:, 3] / 2
    at = tgt[:, 2] * tgt[:, 3]
    ap = pred[:, 2] * pred[:, 3]

    perms, rsus = [], []
    for px in range(NP_SHARD):
        psl = slice(px * P_CORE, (px + 1) * P_CORE)
        perm = _shard_perm(ap[psl])
        perms.append(perm)
        ap_min = ap[psl][perm].reshape(NSLOT, AMB).min(axis=1)   # [NSLOT]
        rsus.append(
            np.float32(1.0)
            / (ap_min[None, :] + at[:, None] + np.float32(EPS))
        )

    in_maps = []
    for c in range(N_CORES):
        px, ty = c % NP_SHARD, c // NP_SHARD
        psl = slice(px * P_CORE, (px + 1) * P_CORE)
        tsl = slice(ty * T_CORE, (ty + 1) * T_CORE)
        perm = perms[px]

        shard = pred[psl][perm]               # permuted pred rows
        x1p, x0p, y1p, y0p = _clip_planes(shard)
        H = P_CORE // 2
        C = 256
        parts = (x1p[:C], x0p[:C], x1p[C:H], x0p[C:H], x1p[H:], x0p[H:],
                 y1p[:H], y0p[:H], y1p[H:], y0p[H:])
        planes = np.empty((128, 4 * P_CORE), np.float16)
        off = 0
        for v in parts:
            planes[:, off : off + v.size] = v.astype(np.float16)[None, :]
            off += v.size

        tsc = np.stack([x1t[tsl], x0t[tsl], y1t[tsl], y0t[tsl],
                        -y0t[tsl], y1t[tsl] - y0t[tsl]])
        tsc = tsc.reshape(6, NJ, 128).transpose(2, 0, 1).reshape(128, 6 * NJ)

        in_maps.append(
            {
                "planes": np.ascontiguousarray(planes),
                "tscal": np.ascontiguousarray(tsc.astype(np.float32)),
            }
        )
    return in_maps, perms, rsus


def _loss_pairs(pred_rows, tgt_rows):
    """Exact reference-form loss for pred_rows[...,4] vs tgt_rows[...,4] f32."""
    p, t = pred_rows, tgt_rows
    x1p = np.minimum(p[..., 0] + p[..., 2] / 2, np.float32(1.0))
    x0p = np.maximum(p[..., 0] - p[..., 2] / 2, np.float32(0.0))
    y1p = np.minimum(p[..., 1] + p[..., 3] / 2, np.float32(1.0))
    y0p = np.maximum(p[..., 1] - p[..., 3] / 2, np.float32(0.0))
    x1t = t[..., 0] + t[..., 2] / 2
    x0t = t[..., 0] - t[..., 2] / 2
    y1t = t[..., 1] + t[..., 3] / 2
    y0t = t[..., 1] - t[..., 3] / 2
    ox0 = np.maximum(x0t, x0p); ox1 = np.minimum(x1t, x1p)
    oy0 = np.maximum(y0t, y0p); oy1 = np.minimum(y1t, y1p)
    nov = (ox1 < ox0) | (oy1 < oy0)
    inter = (ox1 - ox0) * (oy1 - oy0)
    denom = p[..., 2] * p[..., 3] + t[..., 2] * t[..., 3] - inter + np.float32(EPS)
    iou = inter / denom
    d = p - t
    mse = np.sum(d * d, axis=-1) / np.float32(4.0)
    return np.where(nov, np.float32(1.0) + mse, np.float32(1.0) - iou)


def _host_greedy(vals, slots, perms, pred, tgt, rsumax=None, stats=None):
    """vals [T, NSH, 8] f32 desc slot-max bounds; slots [T, NSH, 8] slot ids."""
    # expand: slot s, mate m -> new_pos = s*AMB + m -> local orig via perm
    newpos = (
        slots[..., None] * AMB + np.arange(AMB)[None, None, None, :]
    )  # [T, NSH, 8, AMB]
    g = np.empty(newpos.shape, dtype=np.int64)
    for px in range(NP_SHARD):
        g[:, px] = perms[px][newpos[:, px]] + px * P_CORE
    g = g.reshape(T, -1)
    closs = _loss_pairs(pred[g], tgt[:, None, :]).astype(np.float64)

    order = np.lexsort((g, closs), axis=1)

    v8 = vals[:, :, 7].astype(np.float64)
    u_cap = np.min(v8, axis=1) * 1.03 + 1e-5
    if rsumax is not None:
        # fp16 plane quantization: |corner err| <= 2^-11 -> inter slack
        d = 2.0 ** -11
        u_cap = u_cap + (6 * d + 4 * d * d) * rsumax.astype(np.float64)
    u_cap = np.minimum(u_cap, 0.499999)
    bound = (1.0 - 2.0 * u_cap) / (1.0 - u_cap)

    taken = np.zeros(P_TOTAL, dtype=bool)
    sel = np.empty(T, dtype=np.int64)
    n_fb = 0
    for t in range(T):
        got = -1
        for d in order[t]:
            k = g[t, d]
            if not taken[k]:
                if closs[t, d] <= bound[t] - 1e-6:
                    got = k
                break
        if got < 0:
            n_fb += 1
            row = _loss_pairs(pred, tgt[t][None, :]).astype(np.float64)
            row[taken] = np.inf
            got = int(np.argmin(row))
        taken[got] = True
        sel[t] = got
    if stats is not None:
        stats["fallbacks"] = n_fb
    return np.float32(np.mean(_loss_pairs(pred[sel], tgt).astype(np.float64)))


def kernel(pred_bboxes, target_bboxes):
    from concourse.bass_utils import run_bass_kernel_spmd

    pred = np.asarray(pred_bboxes, dtype=np.float32)[0]
    tgt = np.asarray(target_bboxes, dtype=np.float32)[0]

    if "nc" not in _CACHE:
        _CACHE["nc"] = _build_nc()
    nc = _CACHE["nc"]

    in_maps, perms, rsus = _prep_core_inputs(pred, tgt)
    res = run_bass_kernel_spmd(nc, in_maps, list(range(N_CORES)))
    return _gather_and_reduce(res.results, perms, rsus, pred, tgt)


def _gather_and_reduce(results, perms, rsus, pred, tgt, stats=None):
    # m3[t, px, s]: device slot maxima; kq = m3 * rsu in f32 on host
    m3 = np.empty((T, NP_SHARD, NSLOT), np.float32)
    for c in range(N_CORES):
        px, ty = c % NP_SHARD, c // NP_SHARD
        tsl = slice(ty * T_CORE, (ty + 1) * T_CORE)
        o = results[c]["m3c"].astype(np.float32).reshape(128, NJ, NSLOT)
        m3[tsl, px] = o.transpose(1, 0, 2).reshape(T_CORE, NSLOT)

    kq = m3 * np.stack(rsus, axis=1)          # [T, NP_SHARD, NSLOT]
    part = np.argpartition(-kq, 8, axis=2)[:, :, :8]
    pv = np.take_along_axis(kq, part, axis=2)
    order8 = np.argsort(-pv, axis=2)
    slots = np.take_along_axis(part, order8, axis=2)     # [T, NSH, 8] desc
    vals = np.take_along_axis(pv, order8, axis=2)
    rsumax = np.stack(rsus, axis=1).max(axis=(1, 2))
    return _host_greedy(vals, slots, perms, pred, tgt, rsumax=rsumax, stats=stats)


# revision 36
# speedup vs baseline: 1.0264x; 1.0264x over previous
"""Trainium2 Bass kernel: greedy bbox-matching loss (nn_BboxLoss).

Sharding: 4 pred-shards x 2 target-halves over 8 NeuronCores. Within each
shard, preds are HOST-PERMUTED into area-sorted order so that slot s's 32
mates (area-ranks [32s, 32s+32)) are CONTIGUOUS. Per core, per row-tile j of
[128 targets x 2048 preds]:

  pwx = relu(min(x1p, x1t) - max(x0p, x0t))   [custom DVE op, fp16 in,
  pwy = relu(min(y1p, y1t) - max(y0p, y0t))    bf16 out, 2x perf mode]
  m3  = max over each 32-mate slot of pwx*pwy [custom DVE subdim op, 2x]
  m3 page-max columns compacted on GPSIMD and DMA'd out.

Both custom DVE ops carry hand-authored 2X_1PORT uOp programs (element 1 via
SRC_*_HI, dual WR0_LO/WR0_HI writes — validated against numpy references by
a uop-pipeline simulator) and set perf_max=1 so the engine/cost model run
them at 2 elem/cycle.

The host computes kq = m3 * RSU in f32 (RSU[t, s] = 1/(min-mate-area + at
+ eps)) and takes the top-8 slots per shard. kq[s] upper-bounds
key = inter/S of every mate in slot s (m3 >= inter_p, RSU >= RS_p), and
because mates have adjacent areas it is also a tight estimate, so top-8
slots by kq track the top preds by IoU. Host expands each top-8 slot to
its 32 mates (4 shards x 8 slots x 32 = 1024 candidates per row),
evaluates exact reference losses, and runs the greedy walk.
Each pick is certified sound: non-candidate preds have device-key <= v8
(the 8th slot-max), so their exact loss is floor-bounded via
u_cap = v8*(1+eps_round) + fp16-plane slack; picks that cannot be
certified fall back to an exact full-row argmin. The final loss is the
exact reference-form loss of the selected pairs.
"""
import numpy as np
from contextlib import ExitStack

P_TOTAL = 8192
T = 2048
N_CORES = 8
NP_SHARD = 4
NT_SHARD = 2
P_CORE = P_TOTAL // NP_SHARD   # 2048
T_CORE = T // NT_SHARD         # 1024
NJ = T_CORE // 128             # 8
EPS = 1e-7
NSLOT = 64
AMB = P_CORE // NSLOT          # 32 mates per slot

_CACHE = {}
_PM_PW = 1      # perf_max for the PW op (debug knob)
_PM_IMAX = 1    # perf_max for the IMAX op (debug knob)


# --------------------------------------------------------------------------
# custom DVE ops: PW (with 2x variant) and IMAX (mult + max-over-subdim)
# --------------------------------------------------------------------------
def _pw_2x_uops():
    """relu(min(Src0,C0) - max(Src1,C1)) for the packed (lo, hi) pair.
    lanes: d0=SRC_1, d1=C0, d2=C1, d3=ZERO, d4=SRC_0_HI, d5=SRC_1_HI."""
    from concourse.dve_uop import (
        ENABLE, AluInp, AluOp, DelayInp, InpSel, OutPath, OutSel, Trigger,
        UopConfig, UopDpConfig,
    )

    PREV = AluInp.PREV_ALU_OUT
    D = [AluInp.PREV_DELAY_0 + i for i in range(6)]

    def dp8(nl):
        blocks = []
        for _ in range(8):
            b = UopDpConfig()
            b.pass_through_delay(*range(nl))
            blocks.append(b)
        return blocks

    def cap(b, lane):
        b.delay[lane] = DelayInp.PREV_ALU_OUT
        b.delay_enable[lane] = ENABLE
        return b

    u = UopConfig()
    u.enable_input(InpSel.SRC_0, 0)
    u.enable_input(InpSel.SRC_1, 1)
    u.enable_input(InpSel.CONST_0, 2)
    u.enable_input(InpSel.CONST_1, 3)
    u.enable_input(InpSel.ZERO, 4)
    u.enable_input(InpSel.SRC_0_HI, 5)
    u.enable_input(InpSel.SRC_1_HI, 6)
    dp = dp8(6)
    dp[0].enable_alu(AluOp.MIN, PREV, D[1])           # min_lo
    cap(dp[1].enable_alu(AluOp.MAX, D[0], D[2]), 0)   # max_lo; d0 <- min_lo
    dp[2].enable_alu(AluOp.SUBTRACT, D[0], PREV)      # diff_lo
    dp[3].enable_alu(AluOp.MAX, PREV, D[3])           # pw_lo = relu
    cap(dp[4].enable_alu(AluOp.MIN, D[4], D[1]), 4)   # min_hi; d4 <- pw_lo
    cap(dp[5].enable_alu(AluOp.MAX, D[5], D[2]), 5)   # max_hi; d5 <- min_hi
    dp[6].enable_alu(AluOp.SUBTRACT, D[5], PREV)      # diff_hi
    dp[7].enable_alu(AluOp.MAX, PREV, D[3])           # pw_hi = relu
    u.datapath_config = dp
    u.enable_output(OutSel.DELAY_4, OutPath.WR0_LO)
    u.enable_output(OutSel.ALU_OUT, OutPath.WR0_HI)
    u.require_inp0 = u.require_inp1 = 1
    u.trigger = (Trigger.SRC_TENSOR_DONE, Trigger.NONE, Trigger.NONE)
    u.next_uop = (0, 0, 0)
    return [u]


def _imax_uops(pair):
    """Full-rate running max of (Src0*Src1) with reset at each N-element
    page (SUB_DIM_DONE); the page max lands in the page's LAST output
    element. 3 states: entry-reset, steady, mid-reset (the PageIdx
    step-state idiom). Full-rate writes because write_subdim_last
    misbehaves in 2X_1PORT mode (HW-verified)."""
    from concourse.dve_uop import (
        ENABLE, AluInp, AluOp, DelayInp, InpSel, OutPath, OutSel, Trigger,
        UopConfig, UopDpConfig,
    )

    PREV = AluInp.PREV_ALU_OUT
    CURR = AluInp.CURR_ALU_OUT
    D = [AluInp.PREV_DELAY_0 + i for i in range(6)]
    SRC_DONE, SUBD, CNT = (
        Trigger.SRC_TENSOR_DONE, Trigger.SUB_DIM_DONE, Trigger.COUNT,
    )

    def build(reset):
        u = UopConfig()
        u.enable_input(InpSel.SRC_0, 0)
        u.enable_input(InpSel.SRC_1, 1)          # d0
        nl = 1
        if pair == 2:
            u.enable_input(InpSel.SRC_0_HI, 2)   # d1
            u.enable_input(InpSel.SRC_1_HI, 3)   # d2
            nl = 3
        dp = []
        for _ in range(8):
            b = UopDpConfig()
            b.pass_through_delay(*range(nl))
            dp.append(b)
        if pair == 1:
            dp[0].enable_alu(AluOp.MULTIPLY, PREV, D[0])
            if reset:
                dp[1].enable_alu(AluOp.BYPASS, PREV, PREV)
            else:
                dp[1].enable_alu(AluOp.MAX, CURR, PREV)
            tail = 2
        else:
            dp[0].enable_alu(AluOp.MULTIPLY, PREV, D[0])       # v_lo
            b1 = dp[1].enable_alu(AluOp.MULTIPLY, D[1], D[2])  # v_hi
            b1.delay[0] = DelayInp.PREV_ALU_OUT                # d0 <- v_lo
            b1.delay_enable[0] = ENABLE
            dp[2].enable_alu(AluOp.MAX, D[0], PREV)            # pair max
            if reset:
                dp[3].enable_alu(AluOp.BYPASS, PREV, PREV)
            else:
                dp[3].enable_alu(AluOp.MAX, CURR, PREV)
            tail = 4
        for st in range(tail, 8):
            dp[st].enable_alu(AluOp.BYPASS, PREV, PREV)
        u.datapath_config = dp
        u.enable_output(OutSel.ALU_OUT, OutPath.WR0_LO)
        if pair == 2:
            u.enable_output(OutSel.ALU_OUT, OutPath.WR0_HI)
        u.require_inp0 = u.require_inp1 = 1
        return u

    us = []
    for reset, trig, nxt, rpt in (
        (True, (SRC_DONE, SUBD, CNT), (0, 2, 1), 1),
        (False, (SRC_DONE, SUBD, Trigger.NONE), (0, 2, 0), 0),
        (True, (SRC_DONE, SUBD, CNT), (0, 2, 1), 1),
    ):
        u = build(reset)
        u.trigger = trig
        u.next_uop = nxt
        u.repeat_count = rpt
        us.append(u)
    return us


def _get_dve_ops():
    """Register PW_BBOX_ANT (with 2x variant) and IMAX_SLOT_ANT (1x + 2x,
    subdim reduce). Returns (pw_op, imax_op). The compiled DveOpSpecs are
    seeded into dve_ops._COMPILE_CACHE so dve_table_for_ops writes the
    perf-mode table slots."""
    from concourse.dve_spec import Spec, Src0, Src1, C0, C1, relu, maxx, minn, lower
    from concourse import dve_ops
    from concourse.dve_uop import DveOpSpec

    pw_name, im_name = "PW_BBOX_ANT", "IMAX_SLOT_ANT"
    if pw_name in dve_ops._SUB_OPCODE_FOR_NAME:
        by_name = {o.name: o for o in dve_ops.OPS}
        return by_name[pw_name], by_name[im_name]

    pw_spec = Spec(
        body=relu(minn(Src0, C0) - maxx(Src1, C1)),
        reference=lambda in0, in1, s0, s1, imm2: np.maximum(
            np.minimum(in0.astype(np.float32), s0)
            - np.maximum(in1.astype(np.float32), s1),
            0.0,
        ).astype(np.float32),
    )

    def im_ref(in0, in1, s0, s1, imm2):
        v = in0.astype(np.float32) * in1.astype(np.float32)
        v = v.reshape(v.shape[0], -1, AMB)
        return np.maximum.accumulate(v, axis=-1).reshape(in0.shape)

    im_spec = Spec(body=Src0 * Src1, reference=im_ref)

    row0 = max(dve_ops._SUB_OPCODE_FOR_NAME.values()) + 1
    assert row0 + 1 < 0x20
    out_ops = []
    for name, spec, row, uops_1x, uops_2x, subdim in (
        (pw_name, pw_spec, row0, lower(pw_spec, ver="v3"), _pw_2x_uops(), False),
        (im_name, im_spec, row0 + 1, _imax_uops(1), _imax_uops(2), True),
    ):
        dve_ops._SUB_OPCODE_FOR_NAME[name] = row
        compiled = DveOpSpec(
            name=name, opcode=row, uops=uops_1x, uops_2x=uops_2x,
            rd1_en=True, perf_max=1,
        )
        compiled.validate("v3")
        op = dve_ops.DveOp(name, spec, subdim=subdim, uops_sha={})
        dve_ops.OPS.append(op)
        dve_ops.CUSTOM_DVE_SPECS[name] = spec
        dve_ops._COMPILE_CACHE[(name, "v3")] = compiled
        out_ops.append(op)
    return out_ops[0], out_ops[1]


# --------------------------------------------------------------------------
# device program
# --------------------------------------------------------------------------
def _build_nc():
    import concourse.bacc as bacc
    import concourse.mybir as mybir
    from concourse.tile import TileContext

    f32 = mybir.dt.float32
    bf16 = mybir.dt.bfloat16
    fp16 = mybir.dt.float16
    Alu = mybir.AluOpType
    pw_op, imax_op = _get_dve_ops()

    nc = bacc.Bacc()
    pl_d = nc.dram_tensor("planes", [128, 4 * P_CORE], fp16, kind="ExternalInput")
    tsc_d = nc.dram_tensor("tscal", [128, 6 * NJ], f32, kind="ExternalInput")
    out_d = nc.dram_tensor("m3c", [128, NJ * NSLOT], bf16, kind="ExternalOutput")

    Act = mybir.ActivationFunctionType
    with TileContext(nc) as tc, ExitStack() as ctx:
        const = ctx.enter_context(tc.tile_pool(name="const", bufs=1))
        work = ctx.enter_context(tc.tile_pool(name="work", bufs=4))
        red = ctx.enter_context(tc.tile_pool(name="red", bufs=4))
        actw = ctx.enter_context(tc.tile_pool(name="actw", bufs=3))

        TSC = const.tile([128, 6, NJ], f32)
        XPQ = const.tile([128, 2, P_CORE], fp16)
        YPQ = const.tile([128, 2, P_CORE], fp16)
        X1P, X0P = XPQ[:, 0, :], XPQ[:, 1, :]
        Y1P, Y0P = YPQ[:, 0, :], YPQ[:, 1, :]
        M3C = const.tile([128, NJ, NSLOT], bf16)

        H = P_CORE // 2
        C = 256
        # dram layout: [x1p_c|x0p_c|x1p_r|x0p_r|y1p_h0|y0p_h0|y1p_h1|y0p_h1]
        # (c = first 256 cols, r = rest of h0..h1) — tiny first chunk so the
        # DVE starts ~3.3us in
        # all input DMAs on SP in consumption order — transfers serialize on
        # the shared DMA engines, so issue order IS arrival order
        nc.sync.dma_start(XPQ[:, :, :C], pl_d[:, 0 : 2 * C])
        nc.sync.dma_start(TSC[:].rearrange("p q j -> p (q j)"), tsc_d[:])
        nc.sync.dma_start(XPQ[:, :, C:H], pl_d[:, 2 * C : 2 * H])
        nc.sync.dma_start(YPQ[:, :, :H], pl_d[:, 4 * H : 6 * H])
        nc.sync.dma_start(YPQ[:, :, H:], pl_d[:, 6 * H : 8 * H])
        nc.sync.dma_start(XPQ[:, :, H:], pl_d[:, 2 * H : 4 * H])

        PWX = [None] * NJ
        PWY = [None] * NJ
        M3 = [None] * NJ

        def pw(j, d, slices=(slice(None),)):
            hi = TSC[:, 0 if d == "x" else 2, j : j + 1]
            lo = TSC[:, 1 if d == "x" else 3, j : j + 1]
            P1, P0 = (X1P, X0P) if d == "x" else (Y1P, Y0P)
            if d == "x" and PWX[j] is None or d == "y" and PWY[j] is None:
                dst = work.tile(
                    [128, NSLOT, AMB], bf16, tag=f"pw{d}", name=f"pw{d}{j}"
                )
                if d == "x":
                    PWX[j] = dst
                else:
                    PWY[j] = dst
            dst = PWX[j] if d == "x" else PWY[j]
            flat = dst[:].rearrange("p s n -> p (s n)")
            for sl in slices:
                nc.vector._custom_dve(
                    pw_op, out=flat[:, sl], in0=P1[:, sl], in1=P0[:, sl],
                    s0=hi, s1=lo,
                ).ins.perf_max = _PM_PW

        def imax(j, half=None):
            if M3[j] is None:
                M3[j] = red.tile(
                    [128, NSLOT, AMB], bf16, tag="m3", name=f"m3_{j}"
                )
            hs = slice(None) if half is None else (
                slice(0, NSLOT // 2) if half == 0 else slice(NSLOT // 2, NSLOT)
            )
            nc.vector._custom_dve(
                imax_op, out=M3[j][:, hs], in0=PWX[j][:, hs], in1=PWY[j][:, hs],
                s0=0.0, s1=0.0,
            ).ins.perf_max = _PM_IMAX

        def compact(j):
            # page max sits in each page's last element; gather to [128,64].
            # On DVE (127ns): Pool's in-order queue is owned by the offload
            # adds and would gate every output DMA behind them.
            nc.vector.tensor_scalar(
                M3C[:, j, :], M3[j][:, :, AMB - 1], 1.0, None, op0=Alu.mult
            )
            # late tiles' DMAs from PE: their sem-waits must not block the
            # SP queue that carries the early tiles' (already-ready) DMAs
            eng = nc.scalar if j >= 4 else nc.sync
            eng.dma_start(out_d[:, j * NSLOT : (j + 1) * NSLOT], M3C[:, j, :])

        def compact2(j, half):
            hs = slice(half * NSLOT // 2, (half + 1) * NSLOT // 2)
            nc.vector.tensor_scalar(
                M3C[:, j, hs], M3[j][:, hs, AMB - 1], 1.0, None, op0=Alu.mult
            )
            lo = j * NSLOT + half * NSLOT // 2
            nc.scalar.dma_start(out_d[:, lo : lo + NSLOT // 2], M3C[:, j, hs])

        # pwy for the last OFF tiles runs on ACT+Pool instead of the DVE:
        # pwy = relu(sy - P - Q), P = relu(y1t - y1p), Q = relu(y0p - y0t),
        # sy = y1t - y0t. f32 intermediates: single bf16 rounding at the end,
        # same error budget as the DVE pw path.
        OFF = (5, 6, 7)

        def act_front(j):
            P = actw.tile([128, P_CORE], f32, tag="actp", name=f"actp{j}")
            Q = actw.tile([128, P_CORE], f32, tag="actq", name=f"actq{j}")
            nc.scalar.activation(
                P[:], Y1P[:], Act.Relu, bias=TSC[:, 2, j : j + 1], scale=-1.0
            )
            nc.scalar.activation(
                Q[:], Y0P[:], Act.Relu, bias=TSC[:, 4, j : j + 1], scale=1.0
            )
            return P, Q

        def act_add(j, P, Q):
            Tt = actw.tile([128, P_CORE], f32, tag="actt", name=f"actt{j}")
            nc.gpsimd.tensor_tensor(Tt[:], P[:], Q[:], op=Alu.add)
            return Tt

        def act_back(j, Tt):
            PWY[j] = work.tile(
                [128, NSLOT, AMB], bf16, tag="pwy", name=f"pwy{j}"
            )
            nc.scalar.activation(
                PWY[j][:].rearrange("p s n -> p (s n)"), Tt[:], Act.Relu,
                bias=TSC[:, 5, j : j + 1], scale=-1.0,
            )

        # tiles 0-1 fully split into h0/h1 (pw AND imax halves) so phase-A
        # work needs only the first half-planes — the DVE never waits for
        # x_h1/y_h1
        lo, hi = slice(0, H), slice(H, P_CORE)
        pw(0, "x", (slice(0, C), slice(C, H)))
        pw(1, "x", (lo,))
        pw(0, "y", (lo,))
        imax(0, half=0)
        pw(1, "y", (lo,))
        imax(1, half=0)
        pw(0, "y", (hi,))
        pw(1, "y", (hi,))
        pw(0, "x", (hi,))
        imax(0, half=1)
        pw(1, "x", (hi,))
        imax(1, half=1)
        # ACT front passes for the offloaded tiles, then the Pool adds; the
        # final relus are emitted in order behind them on the ACT queue.
        # high_priority: the scheduler must NOT queue these behind compacts
        # (the pwy chain has a 3-hop latency the late tiles depend on).
        with tc.high_priority():
            PQ = {j: act_front(j) for j in OFF}
            TT = {j: act_add(j, *PQ[j]) for j in OFF}
            for j in OFF:
                act_back(j, TT[j])
        # 2-deep software pipeline: compact trails imax by one tile
        for j in range(2, NJ):
            pw(j, "x")
            if j not in OFF:
                pw(j, "y")
            if j > 2:
                imax(j - 1)
            compact(j - 2)
        # tail: last tile's imax/compact/DMA split in halves so the first
        # half's DMA overlaps the second half's compute
        imax(NJ - 1, half=0)
        compact2(NJ - 1, 0)
        imax(NJ - 1, half=1)
        compact(NJ - 2)
        compact2(NJ - 1, 1)

    # 2x perf mode for the custom ops (uops_2x present in the table;
    # engine falls back to 1x if the mem-pattern does not qualify)
    for b in nc.m.functions[0].blocks:
        for inst in b.instructions:
            if type(inst).__name__ == "InstCustomDveAnt":
                inst.perf_max = (
                    _PM_PW if inst.op_name == "PW_BBOX_ANT" else _PM_IMAX
                )

    nc.compile()
    return nc


# --------------------------------------------------------------------------
# host side
# --------------------------------------------------------------------------
def _clip_planes(pred):
    x1p = np.minimum(pred[:, 0] + pred[:, 2] / 2, np.float32(1.0))
    x0p = np.maximum(pred[:, 0] - pred[:, 2] / 2, np.float32(0.0))
    y1p = np.minimum(pred[:, 1] + pred[:, 3] / 2, np.float32(1.0))
    y0p = np.maximum(pred[:, 1] - pred[:, 3] / 2, np.float32(0.0))
    return x1p, x0p, y1p, y0p


def _shard_perm(ap_shard):
    """Area-sort permutation: position r holds area-rank r, so slot s's
    mates are the contiguous area-ranks [s*AMB, (s+1)*AMB)."""
    return np.argsort(ap_shard, kind="stable")


def _prep_core_inputs(pred, tgt):
    """Build per-core input dicts. pred [P,4], tgt [T,4] float32.
    Returns (in_maps, perms, rsus) with perms[px][new_pos] = local orig idx
    and rsus[px] = f32 [T, NSLOT] = 1/(min-mate-area + at + eps)."""
    x1t = tgt[:, 0] + tgt[:, 2] / 2
    x0t = tgt[:, 0] - tgt[:, 2] / 2
    y1t = tgt[:, 1] + tgt[:, 3] / 2
    y0t = tgt[:, 1] - tgt[:, 3] / 2
    at = tgt[:, 2] * tgt[:, 3]
    ap = pred[:, 2] * pred[:, 3]

    perms, rsus = [], []
    for px in range(NP_SHARD):
        psl = slice(px * P_CORE, (px + 1) * P_CORE)
        perm = _shard_perm(ap[psl])
        perms.append(perm)
        ap_min = ap[psl][perm].reshape(NSLOT, AMB).min(axis=1)   # [NSLOT]
        rsus.append(
            np.float32(1.0)
            / (ap_min[None, :] + at[:, None] + np.float32(EPS))
        )

    in_maps = []
    for c in range(N_CORES):
        px, ty = c % NP_SHARD, c // NP_SHARD
        psl = slice(px * P_CORE, (px + 1) * P_CORE)
        tsl = slice(ty * T_CORE, (ty + 1) * T_CORE)
        perm = perms[px]

        shard = pred[psl][perm]               # permuted pred rows
        x1p, x0p, y1p, y0p = _clip_planes(shard)
        H = P_CORE // 2
        C = 256
        parts = (x1p[:C], x0p[:C], x1p[C:H], x0p[C:H], x1p[H:], x0p[H:],
                 y1p[:H], y0p[:H], y1p[H:], y0p[H:])
        planes = np.empty((128, 4 * P_CORE), np.float16)
        off = 0
        for v in parts:
            planes[:, off : off + v.size] = v.astype(np.float16)[None, :]
            off += v.size

        tsc = np.stack([x1t[tsl], x0t[tsl], y1t[tsl], y0t[tsl],
                        -y0t[tsl], y1t[tsl] - y0t[tsl]])
        tsc = tsc.reshape(6, NJ, 128).transpose(2, 0, 1).reshape(128, 6 * NJ)

        in_maps.append(
            {
                "planes": np.ascontiguousarray(planes),
                "tscal": np.ascontiguousarray(tsc.astype(np.float32)),
            }
        )
    return in_maps, perms, rsus


def _loss_pairs(pred_rows, tgt_rows):
    """Exact reference-form loss for pred_rows[...,4] vs tgt_rows[...,4] f32."""
    p, t = pred_rows, tgt_rows
    x1p = np.minimum(p[..., 0] + p[..., 2] / 2, np.float32(1.0))
    x0p = np.maximum(p[..., 0] - p[..., 2] / 2, np.float32(0.0))
    y1p = np.minimum(p[..., 1] + p[..., 3] / 2, np.float32(1.0))
    y0p = np.maximum(p[..., 1] - p[..., 3] / 2, np.float32(0.0))
    x1t = t[..., 0] + t[..., 2] / 2
    x0t = t[..., 0] - t[..., 2] / 2
    y1t = t[..., 1] + t[..., 3] / 2
    y0t = t[..., 1] - t[..., 3] / 2
    ox0 = np.maximum(x0t, x0p); ox1 = np.minimum(x1t, x1p)
    oy0 = np.maximum(y0t, y0p); oy1 = np.minimum(y1t, y1p)
    nov = (ox1 < ox0) | (oy1 < oy0)
    inter = (ox1 - ox0) * (oy1 - oy0)
    denom = p[..., 2] * p[..., 3] + t[..., 2] * t[..., 3] - inter + np.float32(EPS)
    iou = inter / denom
    d = p - t
    mse = np.sum(d * d, axis=-1) / np.float32(4.0)
    return np.where(nov, np.float32(1.0) + mse, np.float32(1.0) - iou)


def _host_greedy(vals, slots, perms, pred, tgt, rsumax=None, stats=None):
    """vals [T, NSH, 8] f32 desc slot-max bounds; slots [T, NSH, 8] slot ids."""
    # expand: slot s, mate m -> new_pos = s*AMB + m -> local orig via perm
    newpos = (
        slots[..., None] * AMB + np.arange(AMB)[None, None, None, :]
    )  # [T, NSH, 8, AMB]
    g = np.empty(newpos.shape, dtype=np.int64)
    for px in range(NP_SHARD):
        g[:, px] = perms[px][newpos[:, px]] + px * P_CORE
    g = g.reshape(T, -1)
    closs = _loss_pairs(pred[g], tgt[:, None, :]).astype(np.float64)

    order = np.lexsort((g, closs), axis=1)

    v8 = vals[:, :, 7].astype(np.float64)
    u_cap = np.min(v8, axis=1) * 1.03 + 1e-5
    if rsumax is not None:
        # fp16 plane quantization: |corner err| <= 2^-11 -> inter slack
        d = 2.0 ** -11
        u_cap = u_cap + (6 * d + 4 * d * d) * rsumax.astype(np.float64)
    u_cap = np.minimum(u_cap, 0.499999)
    bound = (1.0 - 2.0 * u_cap) / (1.0 - u_cap)

    taken = np.zeros(P_TOTAL, dtype=bool)
    sel = np.empty(T, dtype=np.int64)
    n_fb = 0
    for t in range(T):
        got = -1
        for d in order[t]:
            k = g[t, d]
            if not taken[k]:
                if closs[t, d] <= bound[t] - 1e-6:
                    got = k
                break
        if got < 0:
            n_fb += 1
            row = _loss_pairs(pred, tgt[t][None, :]).astype(np.float64)
            row[taken] = np.inf
            got = int(np.argmin(row))
        taken[got] = True
        sel[t] = got
    if stats is not None:
        stats["fallbacks"] = n_fb
    return np.float32(np.mean(_loss_pairs(pred[sel], tgt).astype(np.float64)))


def kernel(pred_bboxes, target_bboxes):
    from concourse.bass_utils import run_bass_kernel_spmd

    pred = np.asarray(pred_bboxes, dtype=np.float32)[0]
    tgt = np.asarray(target_bboxes, dtype=np.float32)[0]

    if "nc" not in _CACHE:
        _CACHE["nc"] = _build_nc()
    nc = _CACHE["nc"]

    in_maps, perms, rsus = _prep_core_inputs(pred, tgt)
    res = run_bass_kernel_spmd(nc, in_maps, list(range(N_CORES)))
    return _gather_and_reduce(res.results, perms, rsus, pred, tgt)


def _gather_and_reduce(results, perms, rsus, pred, tgt, stats=None):
    # m3[t, px, s]: device slot maxima; kq = m3 * rsu in f32 on host
    m3 = np.empty((T, NP_SHARD, NSLOT), np.float32)
    for c in range(N_CORES):
        px, ty = c % NP_SHARD, c // NP_SHARD
        tsl = slice(ty * T_CORE, (ty + 1) * T_CORE)
        o = results[c]["m3c"].astype(np.float32).reshape(128, NJ, NSLOT)
        m3[tsl, px] = o.transpose(1, 0, 2).reshape(T_CORE, NSLOT)

    kq = m3 * np.stack(rsus, axis=1)          # [T, NP_SHARD, NSLOT]
    part = np.argpartition(-kq, 8, axis=2)[:, :, :8]
    pv = np.take_along_axis(kq, part, axis=2)
    order8 = np.argsort(-pv, axis=2)
    slots = np.take_along_axis(part, order8, axis=2)     # [T, NSH, 8] desc
    vals = np.take_along_axis(pv, order8, axis=2)
    rsumax = np.stack(rsus, axis=1).max(axis=(1, 2))
    return _host_greedy(vals, slots, perms, pred, tgt, rsumax=rsumax, stats=stats)


# revision 37
# speedup vs baseline: 1.0779x; 1.0502x over previous
"""Trainium2 Bass kernel: greedy bbox-matching loss (nn_BboxLoss).

Sharding: 4 pred-shards x 2 target-halves over 8 NeuronCores. Within each
shard, preds are HOST-PERMUTED into area-sorted order so that slot s's 32
mates (area-ranks [32s, 32s+32)) are CONTIGUOUS. Per core, per row-tile j of
[128 targets x 2048 preds]:

  pwx = relu(min(x1p, x1t) - max(x0p, x0t))   [custom DVE op, fp16 in,
  pwy = relu(min(y1p, y1t) - max(y0p, y0t))    bf16 out, 2x perf mode]
  m3  = max over each 32-mate slot of pwx*pwy [custom DVE subdim op, 2x]
  m3 page-max columns compacted on GPSIMD and DMA'd out.

Both custom DVE ops carry hand-authored 2X_1PORT uOp programs (element 1 via
SRC_*_HI, dual WR0_LO/WR0_HI writes — validated against numpy references by
a uop-pipeline simulator) and set perf_max=1 so the engine/cost model run
them at 2 elem/cycle.

The host computes kq = m3 * RSU in f32 (RSU[t, s] = 1/(min-mate-area + at
+ eps)) and takes the top-8 slots per shard. kq[s] upper-bounds
key = inter/S of every mate in slot s (m3 >= inter_p, RSU >= RS_p), and
because mates have adjacent areas it is also a tight estimate, so top-8
slots by kq track the top preds by IoU. Host expands each top-8 slot to
its 32 mates (4 shards x 8 slots x 32 = 1024 candidates per row),
evaluates exact reference losses, and runs the greedy walk.
Each pick is certified sound: non-candidate preds have device-key <= v8
(the 8th slot-max), so their exact loss is floor-bounded via
u_cap = v8*(1+eps_round) + fp16-plane slack; picks that cannot be
certified fall back to an exact full-row argmin. The final loss is the
exact reference-form loss of the selected pairs.
"""
import numpy as np
from contextlib import ExitStack

P_TOTAL = 8192
T = 2048
N_CORES = 8
NP_SHARD = 4
NT_SHARD = 2
P_CORE = P_TOTAL // NP_SHARD   # 2048
T_CORE = T // NT_SHARD         # 1024
NJ = T_CORE // 128             # 8
EPS = 1e-7
NSLOT = 64
AMB = P_CORE // NSLOT          # 32 mates per slot

_CACHE = {}
_PM_PW = 1      # perf_max for the PW op (debug knob)
_PM_IMAX = 1    # perf_max for the IMAX op (debug knob)


# --------------------------------------------------------------------------
# custom DVE ops: PW (with 2x variant) and IMAX (mult + max-over-subdim)
# --------------------------------------------------------------------------
def _pw_2x_uops():
    """relu(min(Src0,C0) - max(Src1,C1)) for the packed (lo, hi) pair.
    lanes: d0=SRC_1, d1=C0, d2=C1, d3=ZERO, d4=SRC_0_HI, d5=SRC_1_HI."""
    from concourse.dve_uop import (
        ENABLE, AluInp, AluOp, DelayInp, InpSel, OutPath, OutSel, Trigger,
        UopConfig, UopDpConfig,
    )

    PREV = AluInp.PREV_ALU_OUT
    D = [AluInp.PREV_DELAY_0 + i for i in range(6)]

    def dp8(nl):
        blocks = []
        for _ in range(8):
            b = UopDpConfig()
            b.pass_through_delay(*range(nl))
            blocks.append(b)
        return blocks

    def cap(b, lane):
        b.delay[lane] = DelayInp.PREV_ALU_OUT
        b.delay_enable[lane] = ENABLE
        return b

    u = UopConfig()
    u.enable_input(InpSel.SRC_0, 0)
    u.enable_input(InpSel.SRC_1, 1)
    u.enable_input(InpSel.CONST_0, 2)
    u.enable_input(InpSel.CONST_1, 3)
    u.enable_input(InpSel.ZERO, 4)
    u.enable_input(InpSel.SRC_0_HI, 5)
    u.enable_input(InpSel.SRC_1_HI, 6)
    dp = dp8(6)
    dp[0].enable_alu(AluOp.MIN, PREV, D[1])           # min_lo
    cap(dp[1].enable_alu(AluOp.MAX, D[0], D[2]), 0)   # max_lo; d0 <- min_lo
    dp[2].enable_alu(AluOp.SUBTRACT, D[0], PREV)      # diff_lo
    dp[3].enable_alu(AluOp.MAX, PREV, D[3])           # pw_lo = relu
    cap(dp[4].enable_alu(AluOp.MIN, D[4], D[1]), 4)   # min_hi; d4 <- pw_lo
    cap(dp[5].enable_alu(AluOp.MAX, D[5], D[2]), 5)   # max_hi; d5 <- min_hi
    dp[6].enable_alu(AluOp.SUBTRACT, D[5], PREV)      # diff_hi
    dp[7].enable_alu(AluOp.MAX, PREV, D[3])           # pw_hi = relu
    u.datapath_config = dp
    u.enable_output(OutSel.DELAY_4, OutPath.WR0_LO)
    u.enable_output(OutSel.ALU_OUT, OutPath.WR0_HI)
    u.require_inp0 = u.require_inp1 = 1
    u.trigger = (Trigger.SRC_TENSOR_DONE, Trigger.NONE, Trigger.NONE)
    u.next_uop = (0, 0, 0)
    return [u]


def _imax_uops(pair):
    """Full-rate running max of (Src0*Src1) with reset at each N-element
    page (SUB_DIM_DONE); the page max lands in the page's LAST output
    element. 3 states: entry-reset, steady, mid-reset (the PageIdx
    step-state idiom). Full-rate writes because write_subdim_last
    misbehaves in 2X_1PORT mode (HW-verified)."""
    from concourse.dve_uop import (
        ENABLE, AluInp, AluOp, DelayInp, InpSel, OutPath, OutSel, Trigger,
        UopConfig, UopDpConfig,
    )

    PREV = AluInp.PREV_ALU_OUT
    CURR = AluInp.CURR_ALU_OUT
    D = [AluInp.PREV_DELAY_0 + i for i in range(6)]
    SRC_DONE, SUBD, CNT = (
        Trigger.SRC_TENSOR_DONE, Trigger.SUB_DIM_DONE, Trigger.COUNT,
    )

    def build(reset):
        u = UopConfig()
        u.enable_input(InpSel.SRC_0, 0)
        u.enable_input(InpSel.SRC_1, 1)          # d0
        nl = 1
        if pair == 2:
            u.enable_input(InpSel.SRC_0_HI, 2)   # d1
            u.enable_input(InpSel.SRC_1_HI, 3)   # d2
            nl = 3
        dp = []
        for _ in range(8):
            b = UopDpConfig()
            b.pass_through_delay(*range(nl))
            dp.append(b)
        if pair == 1:
            dp[0].enable_alu(AluOp.MULTIPLY, PREV, D[0])
            if reset:
                dp[1].enable_alu(AluOp.BYPASS, PREV, PREV)
            else:
                dp[1].enable_alu(AluOp.MAX, CURR, PREV)
            tail = 2
        else:
            dp[0].enable_alu(AluOp.MULTIPLY, PREV, D[0])       # v_lo
            b1 = dp[1].enable_alu(AluOp.MULTIPLY, D[1], D[2])  # v_hi
            b1.delay[0] = DelayInp.PREV_ALU_OUT                # d0 <- v_lo
            b1.delay_enable[0] = ENABLE
            dp[2].enable_alu(AluOp.MAX, D[0], PREV)            # pair max
            if reset:
                dp[3].enable_alu(AluOp.BYPASS, PREV, PREV)
            else:
                dp[3].enable_alu(AluOp.MAX, CURR, PREV)
            tail = 4
        for st in range(tail, 8):
            dp[st].enable_alu(AluOp.BYPASS, PREV, PREV)
        u.datapath_config = dp
        u.enable_output(OutSel.ALU_OUT, OutPath.WR0_LO)
        if pair == 2:
            u.enable_output(OutSel.ALU_OUT, OutPath.WR0_HI)
        u.require_inp0 = u.require_inp1 = 1
        return u

    us = []
    for reset, trig, nxt, rpt in (
        (True, (SRC_DONE, SUBD, CNT), (0, 2, 1), 1),
        (False, (SRC_DONE, SUBD, Trigger.NONE), (0, 2, 0), 0),
        (True, (SRC_DONE, SUBD, CNT), (0, 2, 1), 1),
    ):
        u = build(reset)
        u.trigger = trig
        u.next_uop = nxt
        u.repeat_count = rpt
        us.append(u)
    return us


def _get_dve_ops():
    """Register PW_BBOX_ANT (with 2x variant) and IMAX_SLOT_ANT (1x + 2x,
    subdim reduce). Returns (pw_op, imax_op). The compiled DveOpSpecs are
    seeded into dve_ops._COMPILE_CACHE so dve_table_for_ops writes the
    perf-mode table slots."""
    from concourse.dve_spec import Spec, Src0, Src1, C0, C1, relu, maxx, minn, lower
    from concourse import dve_ops
    from concourse.dve_uop import DveOpSpec

    pw_name, im_name = "PW_BBOX_ANT", "IMAX_SLOT_ANT"
    if pw_name in dve_ops._SUB_OPCODE_FOR_NAME:
        by_name = {o.name: o for o in dve_ops.OPS}
        return by_name[pw_name], by_name[im_name]

    pw_spec = Spec(
        body=relu(minn(Src0, C0) - maxx(Src1, C1)),
        reference=lambda in0, in1, s0, s1, imm2: np.maximum(
            np.minimum(in0.astype(np.float32), s0)
            - np.maximum(in1.astype(np.float32), s1),
            0.0,
        ).astype(np.float32),
    )

    def im_ref(in0, in1, s0, s1, imm2):
        v = in0.astype(np.float32) * in1.astype(np.float32)
        v = v.reshape(v.shape[0], -1, AMB)
        return np.maximum.accumulate(v, axis=-1).reshape(in0.shape)

    im_spec = Spec(body=Src0 * Src1, reference=im_ref)

    row0 = max(dve_ops._SUB_OPCODE_FOR_NAME.values()) + 1
    assert row0 + 1 < 0x20
    out_ops = []
    for name, spec, row, uops_1x, uops_2x, subdim in (
        (pw_name, pw_spec, row0, lower(pw_spec, ver="v3"), _pw_2x_uops(), False),
        (im_name, im_spec, row0 + 1, _imax_uops(1), _imax_uops(2), True),
    ):
        dve_ops._SUB_OPCODE_FOR_NAME[name] = row
        compiled = DveOpSpec(
            name=name, opcode=row, uops=uops_1x, uops_2x=uops_2x,
            rd1_en=True, perf_max=1,
        )
        compiled.validate("v3")
        op = dve_ops.DveOp(name, spec, subdim=subdim, uops_sha={})
        dve_ops.OPS.append(op)
        dve_ops.CUSTOM_DVE_SPECS[name] = spec
        dve_ops._COMPILE_CACHE[(name, "v3")] = compiled
        out_ops.append(op)
    return out_ops[0], out_ops[1]


# --------------------------------------------------------------------------
# device program
# --------------------------------------------------------------------------
def _build_nc():
    import concourse.bacc as bacc
    import concourse.mybir as mybir
    from concourse.tile import TileContext

    f32 = mybir.dt.float32
    bf16 = mybir.dt.bfloat16
    fp16 = mybir.dt.float16
    Alu = mybir.AluOpType
    pw_op, imax_op = _get_dve_ops()

    nc = bacc.Bacc()
    pl_d = nc.dram_tensor("planes", [128, 4 * P_CORE], fp16, kind="ExternalInput")
    tsc_d = nc.dram_tensor("tscal", [128, 6 * NJ], f32, kind="ExternalInput")
    out_d = nc.dram_tensor("m3c", [128, NJ * NSLOT], bf16, kind="ExternalOutput")

    Act = mybir.ActivationFunctionType
    with TileContext(nc) as tc, ExitStack() as ctx:
        const = ctx.enter_context(tc.tile_pool(name="const", bufs=1))
        work = ctx.enter_context(tc.tile_pool(name="work", bufs=4))
        red = ctx.enter_context(tc.tile_pool(name="red", bufs=4))
        actw = ctx.enter_context(tc.tile_pool(name="actw", bufs=3))

        TSC = const.tile([128, 6, NJ], f32)
        XPQ = const.tile([128, 2, P_CORE], fp16)
        YPQ = const.tile([128, 2, P_CORE], fp16)
        X1P, X0P = XPQ[:, 0, :], XPQ[:, 1, :]
        Y1P, Y0P = YPQ[:, 0, :], YPQ[:, 1, :]
        M3C = const.tile([128, NJ, NSLOT], bf16)

        H = P_CORE // 2
        C = 512
        # dram layout: [x1p_c|x0p_c|x1p_r|x0p_r|y1p_h0|y0p_h0|y1p_h1|y0p_h1]
        # (c = first 256 cols, r = rest of h0..h1) — tiny first chunk so the
        # DVE starts ~3.3us in
        # all input DMAs on SP in consumption order — transfers serialize on
        # the shared DMA engines, so issue order IS arrival order
        nc.sync.dma_start(XPQ[:, :, :C], pl_d[:, 0 : 2 * C])
        nc.sync.dma_start(TSC[:].rearrange("p q j -> p (q j)"), tsc_d[:])
        nc.sync.dma_start(XPQ[:, :, C:H], pl_d[:, 2 * C : 2 * H])
        nc.sync.dma_start(YPQ[:, :, :H], pl_d[:, 4 * H : 6 * H])
        nc.sync.dma_start(YPQ[:, :, H:], pl_d[:, 6 * H : 8 * H])
        nc.sync.dma_start(XPQ[:, :, H:], pl_d[:, 2 * H : 4 * H])

        PWX = [None] * NJ
        PWY = [None] * NJ
        M3 = [None] * NJ

        def pw(j, d, slices=(slice(None),)):
            hi = TSC[:, 0 if d == "x" else 2, j : j + 1]
            lo = TSC[:, 1 if d == "x" else 3, j : j + 1]
            P1, P0 = (X1P, X0P) if d == "x" else (Y1P, Y0P)
            if d == "x" and PWX[j] is None or d == "y" and PWY[j] is None:
                dst = work.tile(
                    [128, NSLOT, AMB], bf16, tag=f"pw{d}", name=f"pw{d}{j}"
                )
                if d == "x":
                    PWX[j] = dst
                else:
                    PWY[j] = dst
            dst = PWX[j] if d == "x" else PWY[j]
            flat = dst[:].rearrange("p s n -> p (s n)")
            for sl in slices:
                nc.vector._custom_dve(
                    pw_op, out=flat[:, sl], in0=P1[:, sl], in1=P0[:, sl],
                    s0=hi, s1=lo,
                ).ins.perf_max = _PM_PW

        def imax(j, half=None):
            if M3[j] is None:
                M3[j] = red.tile(
                    [128, NSLOT, AMB], bf16, tag="m3", name=f"m3_{j}"
                )
            hs = slice(None) if half is None else (
                slice(0, NSLOT // 2) if half == 0 else slice(NSLOT // 2, NSLOT)
            )
            nc.vector._custom_dve(
                imax_op, out=M3[j][:, hs], in0=PWX[j][:, hs], in1=PWY[j][:, hs],
                s0=0.0, s1=0.0,
            ).ins.perf_max = _PM_IMAX

        def compact(j, dma=True):
            # page max sits in each page's last element; gather to [128,64].
            # On DVE (127ns): Pool's in-order queue is owned by the offload
            # adds and would gate every output DMA behind them.
            nc.vector.tensor_scalar(
                M3C[:, j, :], M3[j][:, :, AMB - 1], 1.0, None, op0=Alu.mult
            )
            if not dma:
                return
            # late tiles' DMAs from ACT: their sem-waits must not block the
            # SP queue that carries the early tiles' (already-ready) DMAs
            eng = nc.scalar if j >= 4 else nc.sync
            eng.dma_start(out_d[:, j * NSLOT : (j + 1) * NSLOT], M3C[:, j, :])


        # pwy for the last OFF tiles runs on ACT+Pool instead of the DVE:
        # pwy = relu(sy - P - Q), P = relu(y1t - y1p), Q = relu(y0p - y0t),
        # sy = y1t - y0t. f32 intermediates: single bf16 rounding at the end,
        # same error budget as the DVE pw path.
        OFF = (5, 6, 7)

        def act_front(j):
            P = actw.tile([128, P_CORE], f32, tag="actp", name=f"actp{j}")
            Q = actw.tile([128, P_CORE], f32, tag="actq", name=f"actq{j}")
            nc.scalar.activation(
                P[:], Y1P[:], Act.Relu, bias=TSC[:, 2, j : j + 1], scale=-1.0
            )
            nc.scalar.activation(
                Q[:], Y0P[:], Act.Relu, bias=TSC[:, 4, j : j + 1], scale=1.0
            )
            return P, Q

        def act_add(j, P, Q):
            Tt = actw.tile([128, P_CORE], f32, tag="actt", name=f"actt{j}")
            nc.gpsimd.tensor_tensor(Tt[:], P[:], Q[:], op=Alu.add)
            return Tt

        def act_back(j, Tt):
            PWY[j] = work.tile(
                [128, NSLOT, AMB], bf16, tag="pwy", name=f"pwy{j}"
            )
            nc.scalar.activation(
                PWY[j][:].rearrange("p s n -> p (s n)"), Tt[:], Act.Relu,
                bias=TSC[:, 5, j : j + 1], scale=-1.0,
            )

        # tiles 0-1 fully split into h0/h1 (pw AND imax halves) so phase-A
        # work needs only the first half-planes — the DVE never waits for
        # x_h1/y_h1
        lo, hi = slice(0, H), slice(H, P_CORE)
        pw(0, "x", (slice(0, C), slice(C, H)))
        pw(1, "x", (lo,))
        pw(0, "y", (lo,))
        imax(0, half=0)
        pw(1, "y", (lo,))
        imax(1, half=0)
        pw(0, "y", (hi,))
        pw(1, "y", (hi,))
        pw(0, "x", (hi,))
        imax(0, half=1)
        pw(1, "x", (hi,))
        imax(1, half=1)
        # ACT front passes for the offloaded tiles, then the Pool adds; the
        # final relus are emitted in order behind them on the ACT queue.
        # high_priority: the scheduler must NOT queue these behind compacts
        # (the pwy chain has a 3-hop latency the late tiles depend on).
        with tc.high_priority():
            PQ = {j: act_front(j) for j in OFF}
            TT = {j: act_add(j, *PQ[j]) for j in OFF}
            for j in OFF:
                act_back(j, TT[j])
        # 2-deep software pipeline: compact trails imax by one tile
        for j in range(2, NJ):
            pw(j, "x")
            if j not in OFF:
                pw(j, "y")
            if j > 2:
                imax(j - 1)
            compact(j - 2)
        # tail: tiles 6+7 compacted then shipped in ONE final DMA — the
        # output DMAs serialize on HWDGE/DMA engines, so fewer is faster
        imax(NJ - 1)
        compact(NJ - 2, dma=False)
        compact(NJ - 1, dma=False)
        nc.sync.dma_start(
            out_d[:, (NJ - 2) * NSLOT :], M3C[:, NJ - 2 :, :].rearrange(
                "p j s -> p (j s)"
            )
        )

    # 2x perf mode for the custom ops (uops_2x present in the table;
    # engine falls back to 1x if the mem-pattern does not qualify)
    for b in nc.m.functions[0].blocks:
        for inst in b.instructions:
            if type(inst).__name__ == "InstCustomDveAnt":
                inst.perf_max = (
                    _PM_PW if inst.op_name == "PW_BBOX_ANT" else _PM_IMAX
                )

    nc.compile()
    return nc


# --------------------------------------------------------------------------
# host side
# --------------------------------------------------------------------------
def _clip_planes(pred):
    x1p = np.minimum(pred[:, 0] + pred[:, 2] / 2, np.float32(1.0))
    x0p = np.maximum(pred[:, 0] - pred[:, 2] / 2, np.float32(0.0))
    y1p = np.minimum(pred[:, 1] + pred[:, 3] / 2, np.float32(1.0))
    y0p = np.maximum(pred[:, 1] - pred[:, 3] / 2, np.float32(0.0))
    return x1p, x0p, y1p, y0p


def _shard_perm(ap_shard):
    """Area-sort permutation: position r holds area-rank r, so slot s's
    mates are the contiguous area-ranks [s*AMB, (s+1)*AMB)."""
    return np.argsort(ap_shard, kind="stable")


def _prep_core_inputs(pred, tgt):
    """Build per-core input dicts. pred [P,4], tgt [T,4] float32.
    Returns (in_maps, perms, rsus) with perms[px][new_pos] = local orig idx
    and rsus[px] = f32 [T, NSLOT] = 1/(min-mate-area + at + eps)."""
    x1t = tgt[:, 0] + tgt[:, 2] / 2
    x0t = tgt[:, 0] - tgt[:, 2] / 2
    y1t = tgt[:, 1] + tgt[:, 3] / 2
    y0t = tgt[:, 1] - tgt[:, 3] / 2
    at = tgt[:, 2] * tgt[:, 3]
    ap = pred[:, 2] * pred[:, 3]

    perms, rsus = [], []
    for px in range(NP_SHARD):
        psl = slice(px * P_CORE, (px + 1) * P_CORE)
        perm = _shard_perm(ap[psl])
        perms.append(perm)
        ap_min = ap[psl][perm].reshape(NSLOT, AMB).min(axis=1)   # [NSLOT]
        rsus.append(
            np.float32(1.0)
            / (ap_min[None, :] + at[:, None] + np.float32(EPS))
        )

    in_maps = []
    for c in range(N_CORES):
        px, ty = c % NP_SHARD, c // NP_SHARD
        psl = slice(px * P_CORE, (px + 1) * P_CORE)
        tsl = slice(ty * T_CORE, (ty + 1) * T_CORE)
        perm = perms[px]

        shard = pred[psl][perm]               # permuted pred rows
        x1p, x0p, y1p, y0p = _clip_planes(shard)
        H = P_CORE // 2
        C = 512
        parts = (x1p[:C], x0p[:C], x1p[C:H], x0p[C:H], x1p[H:], x0p[H:],
                 y1p[:H], y0p[:H], y1p[H:], y0p[H:])
        planes = np.empty((128, 4 * P_CORE), np.float16)
        off = 0
        for v in parts:
            planes[:, off : off + v.size] = v.astype(np.float16)[None, :]
            off += v.size

        tsc = np.stack([x1t[tsl], x0t[tsl], y1t[tsl], y0t[tsl],
                        -y0t[tsl], y1t[tsl] - y0t[tsl]])
        tsc = tsc.reshape(6, NJ, 128).transpose(2, 0, 1).reshape(128, 6 * NJ)

        in_maps.append(
            {
                "planes": np.ascontiguousarray(planes),
                "tscal": np.ascontiguousarray(tsc.astype(np.float32)),
            }
        )
    return in_maps, perms, rsus


def _loss_pairs(pred_rows, tgt_rows):
    """Exact reference-form loss for pred_rows[...,4] vs tgt_rows[...,4] f32."""
    p, t = pred_rows, tgt_rows
    x1p = np.minimum(p[..., 0] + p[..., 2] / 2, np.float32(1.0))
    x0p = np.maximum(p[..., 0] - p[..., 2] / 2, np.float32(0.0))
    y1p = np.minimum(p[..., 1] + p[..., 3] / 2, np.float32(1.0))
    y0p = np.maximum(p[..., 1] - p[..., 3] / 2, np.float32(0.0))
    x1t = t[..., 0] + t[..., 2] / 2
    x0t = t[..., 0] - t[..., 2] / 2
    y1t = t[..., 1] + t[..., 3] / 2
    y0t = t[..., 1] - t[..., 3] / 2
    ox0 = np.maximum(x0t, x0p); ox1 = np.minimum(x1t, x1p)
    oy0 = np.maximum(y0t, y0p); oy1 = np.minimum(y1t, y1p)
    nov = (ox1 < ox0) | (oy1 < oy0)
    inter = (ox1 - ox0) * (oy1 - oy0)
    denom = p[..., 2] * p[..., 3] + t[..., 2] * t[..., 3] - inter + np.float32(EPS)
    iou = inter / denom
    d = p - t
    mse = np.sum(d * d, axis=-1) / np.float32(4.0)
    return np.where(nov, np.float32(1.0) + mse, np.float32(1.0) - iou)


def _host_greedy(vals, slots, perms, pred, tgt, rsumax=None, stats=None):
    """vals [T, NSH, 8] f32 desc slot-max bounds; slots [T, NSH, 8] slot ids."""
    # expand: slot s, mate m -> new_pos = s*AMB + m -> local orig via perm
    newpos = (
        slots[..., None] * AMB + np.arange(AMB)[None, None, None, :]
    )  # [T, NSH, 8, AMB]
    g = np.empty(newpos.shape, dtype=np.int64)
    for px in range(NP_SHARD):
        g[:, px] = perms[px][newpos[:, px]] + px * P_CORE
    g = g.reshape(T, -1)
    closs = _loss_pairs(pred[g], tgt[:, None, :]).astype(np.float64)

    order = np.lexsort((g, closs), axis=1)

    v8 = vals[:, :, 7].astype(np.float64)
    u_cap = np.min(v8, axis=1) * 1.03 + 1e-5
    if rsumax is not None:
        # fp16 plane quantization: |corner err| <= 2^-11 -> inter slack
        d = 2.0 ** -11
        u_cap = u_cap + (6 * d + 4 * d * d) * rsumax.astype(np.float64)
    u_cap = np.minimum(u_cap, 0.499999)
    bound = (1.0 - 2.0 * u_cap) / (1.0 - u_cap)

    taken = np.zeros(P_TOTAL, dtype=bool)
    sel = np.empty(T, dtype=np.int64)
    n_fb = 0
    for t in range(T):
        got = -1
        for d in order[t]:
            k = g[t, d]
            if not taken[k]:
                if closs[t, d] <= bound[t] - 1e-6:
                    got = k
                break
        if got < 0:
            n_fb += 1
            row = _loss_pairs(pred, tgt[t][None, :]).astype(np.float64)
            row[taken] = np.inf
            got = int(np.argmin(row))
        taken[got] = True
        sel[t] = got
    if stats is not None:
        stats["fallbacks"] = n_fb
    return np.float32(np.mean(_loss_pairs(pred[sel], tgt).astype(np.float64)))


def kernel(pred_bboxes, target_bboxes):
    from concourse.bass_utils import run_bass_kernel_spmd

    pred = np.asarray(pred_bboxes, dtype=np.float32)[0]
    tgt = np.asarray(target_bboxes, dtype=np.float32)[0]

    if "nc" not in _CACHE:
        _CACHE["nc"] = _build_nc()
    nc = _CACHE["nc"]

    in_maps, perms, rsus = _prep_core_inputs(pred, tgt)
    res = run_bass_kernel_spmd(nc, in_maps, list(range(N_CORES)))
    return _gather_and_reduce(res.results, perms, rsus, pred, tgt)


def _gather_and_reduce(results, perms, rsus, pred, tgt, stats=None):
    # m3[t, px, s]: device slot maxima; kq = m3 * rsu in f32 on host
    m3 = np.empty((T, NP_SHARD, NSLOT), np.float32)
    for c in range(N_CORES):
        px, ty = c % NP_SHARD, c // NP_SHARD
        tsl = slice(ty * T_CORE, (ty + 1) * T_CORE)
        o = results[c]["m3c"].astype(np.float32).reshape(128, NJ, NSLOT)
        m3[tsl, px] = o.transpose(1, 0, 2).reshape(T_CORE, NSLOT)

    kq = m3 * np.stack(rsus, axis=1)          # [T, NP_SHARD, NSLOT]
    part = np.argpartition(-kq, 8, axis=2)[:, :, :8]
    pv = np.take_along_axis(kq, part, axis=2)
    order8 = np.argsort(-pv, axis=2)
    slots = np.take_along_axis(part, order8, axis=2)     # [T, NSH, 8] desc
    vals = np.take_along_axis(pv, order8, axis=2)
    rsumax = np.stack(rsus, axis=1).max(axis=(1, 2))
    return _host_greedy(vals, slots, perms, pred, tgt, rsumax=rsumax, stats=stats)


# revision 38
# speedup vs baseline: 1.0882x; 1.0096x over previous
"""Trainium2 Bass kernel: greedy bbox-matching loss (nn_BboxLoss).

Sharding: 4 pred-shards x 2 target-halves over 8 NeuronCores. Within each
shard, preds are HOST-PERMUTED into area-sorted order so that slot s's 32
mates (area-ranks [32s, 32s+32)) are CONTIGUOUS. Per core, per row-tile j of
[128 targets x 2048 preds]:

  pwx = relu(min(x1p, x1t) - max(x0p, x0t))   [custom DVE op, fp16 in,
  pwy = relu(min(y1p, y1t) - max(y0p, y0t))    bf16 out, 2x perf mode]
  m3  = max over each 32-mate slot of pwx*pwy [custom DVE subdim op, 2x]
  m3 page-max columns compacted on the DVE and DMA'd out.

Both custom DVE ops carry hand-authored 2X_1PORT uOp programs (element 1 via
SRC_*_HI, dual WR0_LO/WR0_HI writes — validated against numpy references by
a uop-pipeline simulator) and set perf_max=1 so the engine/cost model run
them at 2 elem/cycle.

The host computes kq = m3 * RSU in f32 (RSU[t, s] = 1/(min-mate-area + at
+ eps)) and takes the top-8 slots per shard. kq[s] upper-bounds
key = inter/S of every mate in slot s (m3 >= inter_p, RSU >= RS_p), and
because mates have adjacent areas it is also a tight estimate, so top-8
slots by kq track the top preds by IoU. Host expands each top-8 slot to
its 32 mates (4 shards x 8 slots x 32 = 1024 candidates per row),
evaluates exact reference losses, and runs the greedy walk.
Each pick is certified sound: non-candidate preds have device-key <= v8
(the 8th slot-max), so their exact loss is floor-bounded via
u_cap = v8*(1+eps_round) + fp16-plane slack; picks that cannot be
certified fall back to an exact full-row argmin. The final loss is the
exact reference-form loss of the selected pairs.
"""
import numpy as np
from contextlib import ExitStack

P_TOTAL = 8192
T = 2048
N_CORES = 8
NP_SHARD = 4
NT_SHARD = 2
P_CORE = P_TOTAL // NP_SHARD   # 2048
T_CORE = T // NT_SHARD         # 1024
NJ = T_CORE // 128             # 8
EPS = 1e-7
NSLOT = 64
AMB = P_CORE // NSLOT          # 32 mates per slot

_CACHE = {}
_PM_PW = 1      # perf_max for the PW op (debug knob)
_PM_IMAX = 1    # perf_max for the IMAX op (debug knob)


# --------------------------------------------------------------------------
# custom DVE ops: PW (with 2x variant) and IMAX (mult + max-over-subdim)
# --------------------------------------------------------------------------
def _pw_2x_uops():
    """relu(min(Src0,C0) - max(Src1,C1)) for the packed (lo, hi) pair.
    lanes: d0=SRC_1, d1=C0, d2=C1, d3=ZERO, d4=SRC_0_HI, d5=SRC_1_HI."""
    from concourse.dve_uop import (
        ENABLE, AluInp, AluOp, DelayInp, InpSel, OutPath, OutSel, Trigger,
        UopConfig, UopDpConfig,
    )

    PREV = AluInp.PREV_ALU_OUT
    D = [AluInp.PREV_DELAY_0 + i for i in range(6)]

    def dp8(nl):
        blocks = []
        for _ in range(8):
            b = UopDpConfig()
            b.pass_through_delay(*range(nl))
            blocks.append(b)
        return blocks

    def cap(b, lane):
        b.delay[lane] = DelayInp.PREV_ALU_OUT
        b.delay_enable[lane] = ENABLE
        return b

    u = UopConfig()
    u.enable_input(InpSel.SRC_0, 0)
    u.enable_input(InpSel.SRC_1, 1)
    u.enable_input(InpSel.CONST_0, 2)
    u.enable_input(InpSel.CONST_1, 3)
    u.enable_input(InpSel.ZERO, 4)
    u.enable_input(InpSel.SRC_0_HI, 5)
    u.enable_input(InpSel.SRC_1_HI, 6)
    dp = dp8(6)
    dp[0].enable_alu(AluOp.MIN, PREV, D[1])           # min_lo
    cap(dp[1].enable_alu(AluOp.MAX, D[0], D[2]), 0)   # max_lo; d0 <- min_lo
    dp[2].enable_alu(AluOp.SUBTRACT, D[0], PREV)      # diff_lo
    dp[3].enable_alu(AluOp.MAX, PREV, D[3])           # pw_lo = relu
    cap(dp[4].enable_alu(AluOp.MIN, D[4], D[1]), 4)   # min_hi; d4 <- pw_lo
    cap(dp[5].enable_alu(AluOp.MAX, D[5], D[2]), 5)   # max_hi; d5 <- min_hi
    dp[6].enable_alu(AluOp.SUBTRACT, D[5], PREV)      # diff_hi
    dp[7].enable_alu(AluOp.MAX, PREV, D[3])           # pw_hi = relu
    u.datapath_config = dp
    u.enable_output(OutSel.DELAY_4, OutPath.WR0_LO)
    u.enable_output(OutSel.ALU_OUT, OutPath.WR0_HI)
    u.require_inp0 = u.require_inp1 = 1
    u.trigger = (Trigger.SRC_TENSOR_DONE, Trigger.NONE, Trigger.NONE)
    u.next_uop = (0, 0, 0)
    return [u]


def _imax_uops(pair):
    """Full-rate running max of (Src0*Src1) with reset at each N-element
    page (SUB_DIM_DONE); the page max lands in the page's LAST output
    element. 3 states: entry-reset, steady, mid-reset (the PageIdx
    step-state idiom). Full-rate writes because write_subdim_last
    misbehaves in 2X_1PORT mode (HW-verified)."""
    from concourse.dve_uop import (
        ENABLE, AluInp, AluOp, DelayInp, InpSel, OutPath, OutSel, Trigger,
        UopConfig, UopDpConfig,
    )

    PREV = AluInp.PREV_ALU_OUT
    CURR = AluInp.CURR_ALU_OUT
    D = [AluInp.PREV_DELAY_0 + i for i in range(6)]
    SRC_DONE, SUBD, CNT = (
        Trigger.SRC_TENSOR_DONE, Trigger.SUB_DIM_DONE, Trigger.COUNT,
    )

    def build(reset):
        u = UopConfig()
        u.enable_input(InpSel.SRC_0, 0)
        u.enable_input(InpSel.SRC_1, 1)          # d0
        nl = 1
        if pair == 2:
            u.enable_input(InpSel.SRC_0_HI, 2)   # d1
            u.enable_input(InpSel.SRC_1_HI, 3)   # d2
            nl = 3
        dp = []
        for _ in range(8):
            b = UopDpConfig()
            b.pass_through_delay(*range(nl))
            dp.append(b)
        if pair == 1:
            dp[0].enable_alu(AluOp.MULTIPLY, PREV, D[0])
            if reset:
                dp[1].enable_alu(AluOp.BYPASS, PREV, PREV)
            else:
                dp[1].enable_alu(AluOp.MAX, CURR, PREV)
            tail = 2
        else:
            dp[0].enable_alu(AluOp.MULTIPLY, PREV, D[0])       # v_lo
            b1 = dp[1].enable_alu(AluOp.MULTIPLY, D[1], D[2])  # v_hi
            b1.delay[0] = DelayInp.PREV_ALU_OUT                # d0 <- v_lo
            b1.delay_enable[0] = ENABLE
            dp[2].enable_alu(AluOp.MAX, D[0], PREV)            # pair max
            if reset:
                dp[3].enable_alu(AluOp.BYPASS, PREV, PREV)
            else:
                dp[3].enable_alu(AluOp.MAX, CURR, PREV)
            tail = 4
        for st in range(tail, 8):
            dp[st].enable_alu(AluOp.BYPASS, PREV, PREV)
        u.datapath_config = dp
        u.enable_output(OutSel.ALU_OUT, OutPath.WR0_LO)
        if pair == 2:
            u.enable_output(OutSel.ALU_OUT, OutPath.WR0_HI)
        u.require_inp0 = u.require_inp1 = 1
        return u

    us = []
    for reset, trig, nxt, rpt in (
        (True, (SRC_DONE, SUBD, CNT), (0, 2, 1), 1),
        (False, (SRC_DONE, SUBD, Trigger.NONE), (0, 2, 0), 0),
        (True, (SRC_DONE, SUBD, CNT), (0, 2, 1), 1),
    ):
        u = build(reset)
        u.trigger = trig
        u.next_uop = nxt
        u.repeat_count = rpt
        us.append(u)
    return us


def _get_dve_ops():
    """Register PW_BBOX_ANT (with 2x variant) and IMAX_SLOT_ANT (1x + 2x,
    subdim reduce). Returns (pw_op, imax_op). The compiled DveOpSpecs are
    seeded into dve_ops._COMPILE_CACHE so dve_table_for_ops writes the
    perf-mode table slots."""
    from concourse.dve_spec import Spec, Src0, Src1, C0, C1, relu, maxx, minn, lower
    from concourse import dve_ops
    from concourse.dve_uop import DveOpSpec

    pw_name, im_name = "PW_BBOX_ANT", "IMAX_SLOT_ANT"
    if pw_name in dve_ops._SUB_OPCODE_FOR_NAME:
        by_name = {o.name: o for o in dve_ops.OPS}
        return by_name[pw_name], by_name[im_name]

    pw_spec = Spec(
        body=relu(minn(Src0, C0) - maxx(Src1, C1)),
        reference=lambda in0, in1, s0, s1, imm2: np.maximum(
            np.minimum(in0.astype(np.float32), s0)
            - np.maximum(in1.astype(np.float32), s1),
            0.0,
        ).astype(np.float32),
    )

    def im_ref(in0, in1, s0, s1, imm2):
        v = in0.astype(np.float32) * in1.astype(np.float32)
        v = v.reshape(v.shape[0], -1, AMB)
        return np.maximum.accumulate(v, axis=-1).reshape(in0.shape)

    im_spec = Spec(body=Src0 * Src1, reference=im_ref)

    row0 = max(dve_ops._SUB_OPCODE_FOR_NAME.values()) + 1
    assert row0 + 1 < 0x20
    out_ops = []
    for name, spec, row, uops_1x, uops_2x, subdim in (
        (pw_name, pw_spec, row0, lower(pw_spec, ver="v3"), _pw_2x_uops(), False),
        (im_name, im_spec, row0 + 1, _imax_uops(1), _imax_uops(2), True),
    ):
        dve_ops._SUB_OPCODE_FOR_NAME[name] = row
        compiled = DveOpSpec(
            name=name, opcode=row, uops=uops_1x, uops_2x=uops_2x,
            rd1_en=True, perf_max=1,
        )
        compiled.validate("v3")
        op = dve_ops.DveOp(name, spec, subdim=subdim, uops_sha={})
        dve_ops.OPS.append(op)
        dve_ops.CUSTOM_DVE_SPECS[name] = spec
        dve_ops._COMPILE_CACHE[(name, "v3")] = compiled
        out_ops.append(op)
    return out_ops[0], out_ops[1]


# --------------------------------------------------------------------------
# device program
# --------------------------------------------------------------------------
def _build_nc():
    import concourse.bacc as bacc
    import concourse.mybir as mybir
    from concourse.tile import TileContext

    f32 = mybir.dt.float32
    bf16 = mybir.dt.bfloat16
    fp16 = mybir.dt.float16
    Alu = mybir.AluOpType
    pw_op, imax_op = _get_dve_ops()

    nc = bacc.Bacc()
    pl_d = nc.dram_tensor("planes", [128, 4 * P_CORE], fp16, kind="ExternalInput")
    tsc_d = nc.dram_tensor("tscal", [128, 6 * NJ], f32, kind="ExternalInput")
    out_d = nc.dram_tensor("m3c", [128, NJ * NSLOT], bf16, kind="ExternalOutput")

    Act = mybir.ActivationFunctionType
    with TileContext(nc) as tc, ExitStack() as ctx:
        const = ctx.enter_context(tc.tile_pool(name="const", bufs=1))
        work = ctx.enter_context(tc.tile_pool(name="work", bufs=4))
        red = ctx.enter_context(tc.tile_pool(name="red", bufs=4))
        actw = ctx.enter_context(tc.tile_pool(name="actw", bufs=3))

        TSC = const.tile([128, 6, NJ], f32)
        XPQ = const.tile([128, 2, P_CORE], fp16)
        YPQ = const.tile([128, 2, P_CORE], fp16)
        X1P, X0P = XPQ[:, 0, :], XPQ[:, 1, :]
        Y1P, Y0P = YPQ[:, 0, :], YPQ[:, 1, :]
        M3C = const.tile([128, NJ, NSLOT], bf16)

        H = P_CORE // 2
        C = 512
        # dram layout: [x1p_c|x0p_c|x1p_r|x0p_r|y1p_h0|y0p_h0|y1p_h1|y0p_h1]
        # (c = first 256 cols, r = rest of h0..h1) — tiny first chunk so the
        # DVE starts ~3.3us in
        # all input DMAs on SP in consumption order — transfers serialize on
        # the shared DMA engines, so issue order IS arrival order
        nc.sync.dma_start(XPQ[:, :, :C], pl_d[:, 0 : 2 * C])
        nc.sync.dma_start(TSC[:].rearrange("p q j -> p (q j)"), tsc_d[:])
        nc.sync.dma_start(XPQ[:, :, C:H], pl_d[:, 2 * C : 2 * H])
        nc.sync.dma_start(YPQ[:, :, :C], pl_d[:, 4 * H : 4 * H + 2 * C])
        nc.sync.dma_start(YPQ[:, :, C:H], pl_d[:, 4 * H + 2 * C : 6 * H])
        nc.sync.dma_start(YPQ[:, :, H:], pl_d[:, 6 * H : 8 * H])
        nc.sync.dma_start(XPQ[:, :, H:], pl_d[:, 2 * H : 4 * H])

        PWX = [None] * NJ
        PWY = [None] * NJ
        M3 = [None] * NJ

        def pw(j, d, slices=(slice(None),)):
            hi = TSC[:, 0 if d == "x" else 2, j : j + 1]
            lo = TSC[:, 1 if d == "x" else 3, j : j + 1]
            P1, P0 = (X1P, X0P) if d == "x" else (Y1P, Y0P)
            if d == "x" and PWX[j] is None or d == "y" and PWY[j] is None:
                dst = work.tile(
                    [128, NSLOT, AMB], bf16, tag=f"pw{d}", name=f"pw{d}{j}"
                )
                if d == "x":
                    PWX[j] = dst
                else:
                    PWY[j] = dst
            dst = PWX[j] if d == "x" else PWY[j]
            flat = dst[:].rearrange("p s n -> p (s n)")
            for sl in slices:
                nc.vector._custom_dve(
                    pw_op, out=flat[:, sl], in0=P1[:, sl], in1=P0[:, sl],
                    s0=hi, s1=lo,
                ).ins.perf_max = _PM_PW

        def imax(j, half=None):
            if M3[j] is None:
                M3[j] = red.tile(
                    [128, NSLOT, AMB], bf16, tag="m3", name=f"m3_{j}"
                )
            hs = slice(None) if half is None else (
                slice(0, NSLOT // 2) if half == 0 else slice(NSLOT // 2, NSLOT)
            )
            nc.vector._custom_dve(
                imax_op, out=M3[j][:, hs], in0=PWX[j][:, hs], in1=PWY[j][:, hs],
                s0=0.0, s1=0.0,
            ).ins.perf_max = _PM_IMAX

        def compact(j, dma=True):
            # page max sits in each page's last element; gather to [128,64].
            # On DVE (127ns): Pool's in-order queue is owned by the offload
            # adds and would gate every output DMA behind them.
            nc.vector.tensor_scalar(
                M3C[:, j, :], M3[j][:, :, AMB - 1], 1.0, None, op0=Alu.mult
            )
            if not dma:
                return
            # late tiles' DMAs from ACT: their sem-waits must not block the
            # SP queue that carries the early tiles' (already-ready) DMAs
            eng = nc.scalar if j >= 4 else nc.sync
            eng.dma_start(out_d[:, j * NSLOT : (j + 1) * NSLOT], M3C[:, j, :])


        # pwy for the last OFF tiles runs on ACT+Pool instead of the DVE:
        # pwy = relu(sy - P - Q), P = relu(y1t - y1p), Q = relu(y0p - y0t),
        # sy = y1t - y0t. f32 intermediates: single bf16 rounding at the end,
        # same error budget as the DVE pw path.
        OFF = (5, 6, 7)

        def act_front(j):
            P = actw.tile([128, P_CORE], f32, tag="actp", name=f"actp{j}")
            Q = actw.tile([128, P_CORE], f32, tag="actq", name=f"actq{j}")
            nc.scalar.activation(
                P[:], Y1P[:], Act.Relu, bias=TSC[:, 2, j : j + 1], scale=-1.0
            )
            nc.scalar.activation(
                Q[:], Y0P[:], Act.Relu, bias=TSC[:, 4, j : j + 1], scale=1.0
            )
            return P, Q

        def act_add(j, P, Q):
            Tt = actw.tile([128, P_CORE], f32, tag="actt", name=f"actt{j}")
            nc.gpsimd.tensor_tensor(Tt[:], P[:], Q[:], op=Alu.add)
            return Tt

        def act_back(j, Tt):
            PWY[j] = work.tile(
                [128, NSLOT, AMB], bf16, tag="pwy", name=f"pwy{j}"
            )
            nc.scalar.activation(
                PWY[j][:].rearrange("p s n -> p (s n)"), Tt[:], Act.Relu,
                bias=TSC[:, 5, j : j + 1], scale=-1.0,
            )

        # tiles 0-1 fully split into h0/h1 (pw AND imax halves) so phase-A
        # work needs only the first half-planes — the DVE never waits for
        # x_h1/y_h1
        lo, hi = slice(0, H), slice(H, P_CORE)
        pw(0, "x", (slice(0, C), slice(C, H)))
        pw(0, "y", (slice(0, C), slice(C, H)))
        pw(1, "x", (lo,))
        imax(0, half=0)
        pw(1, "y", (lo,))
        imax(1, half=0)
        pw(0, "y", (hi,))
        pw(1, "y", (hi,))
        pw(0, "x", (hi,))
        imax(0, half=1)
        pw(1, "x", (hi,))
        imax(1, half=1)
        # ACT front passes for the offloaded tiles, then the Pool adds; the
        # final relus are emitted in order behind them on the ACT queue.
        # high_priority: the scheduler must NOT queue these behind compacts
        # (the pwy chain has a 3-hop latency the late tiles depend on).
        with tc.high_priority():
            PQ = {j: act_front(j) for j in OFF}
            TT = {j: act_add(j, *PQ[j]) for j in OFF}
            for j in OFF:
                act_back(j, TT[j])
        # 2-deep software pipeline: compact trails imax by one tile
        for j in range(2, NJ):
            pw(j, "x")
            if j not in OFF:
                pw(j, "y")
            if j > 2:
                imax(j - 1)
            compact(j - 2)
        # tail: tiles 6+7 compacted then shipped in ONE final DMA — the
        # output DMAs serialize on HWDGE/DMA engines, so fewer is faster
        imax(NJ - 1)
        compact(NJ - 2, dma=False)
        compact(NJ - 1, dma=False)
        nc.sync.dma_start(
            out_d[:, (NJ - 2) * NSLOT :], M3C[:, NJ - 2 :, :].rearrange(
                "p j s -> p (j s)"
            )
        )

    # 2x perf mode for the custom ops (uops_2x present in the table;
    # engine falls back to 1x if the mem-pattern does not qualify)
    for b in nc.m.functions[0].blocks:
        for inst in b.instructions:
            if type(inst).__name__ == "InstCustomDveAnt":
                inst.perf_max = (
                    _PM_PW if inst.op_name == "PW_BBOX_ANT" else _PM_IMAX
                )

    nc.compile()
    return nc


# --------------------------------------------------------------------------
# host side
# --------------------------------------------------------------------------
def _clip_planes(pred):
    x1p = np.minimum(pred[:, 0] + pred[:, 2] / 2, np.float32(1.0))
    x0p = np.maximum(pred[:, 0] - pred[:, 2] / 2, np.float32(0.0))
    y1p = np.minimum(pred[:, 1] + pred[:, 3] / 2, np.float32(1.0))
    y0p = np.maximum(pred[:, 1] - pred[:, 3] / 2, np.float32(0.0))
    return x1p, x0p, y1p, y0p


def _shard_perm(ap_shard):
    """Area-sort permutation: position r holds area-rank r, so slot s's
    mates are the contiguous area-ranks [s*AMB, (s+1)*AMB)."""
    return np.argsort(ap_shard, kind="stable")


def _prep_core_inputs(pred, tgt):
    """Build per-core input dicts. pred [P,4], tgt [T,4] float32.
    Returns (in_maps, perms, rsus) with perms[px][new_pos] = local orig idx
    and rsus[px] = f32 [T, NSLOT] = 1/(min-mate-area + at + eps)."""
    x1t = tgt[:, 0] + tgt[:, 2] / 2
    x0t = tgt[:, 0] - tgt[:, 2] / 2
    y1t = tgt[:, 1] + tgt[:, 3] / 2
    y0t = tgt[:, 1] - tgt[:, 3] / 2
    at = tgt[:, 2] * tgt[:, 3]
    ap = pred[:, 2] * pred[:, 3]

    perms, rsus = [], []
    for px in range(NP_SHARD):
        psl = slice(px * P_CORE, (px + 1) * P_CORE)
        perm = _shard_perm(ap[psl])
        perms.append(perm)
        ap_min = ap[psl][perm].reshape(NSLOT, AMB).min(axis=1)   # [NSLOT]
        rsus.append(
            np.float32(1.0)
            / (ap_min[None, :] + at[:, None] + np.float32(EPS))
        )

    in_maps = []
    for c in range(N_CORES):
        px, ty = c % NP_SHARD, c // NP_SHARD
        psl = slice(px * P_CORE, (px + 1) * P_CORE)
        tsl = slice(ty * T_CORE, (ty + 1) * T_CORE)
        perm = perms[px]

        shard = pred[psl][perm]               # permuted pred rows
        x1p, x0p, y1p, y0p = _clip_planes(shard)
        H = P_CORE // 2
        C = 512
        parts = (x1p[:C], x0p[:C], x1p[C:H], x0p[C:H], x1p[H:], x0p[H:],
                 y1p[:C], y0p[:C], y1p[C:H], y0p[C:H], y1p[H:], y0p[H:])
        planes = np.empty((128, 4 * P_CORE), np.float16)
        off = 0
        for v in parts:
            planes[:, off : off + v.size] = v.astype(np.float16)[None, :]
            off += v.size

        tsc = np.stack([x1t[tsl], x0t[tsl], y1t[tsl], y0t[tsl],
                        -y0t[tsl], y1t[tsl] - y0t[tsl]])
        tsc = tsc.reshape(6, NJ, 128).transpose(2, 0, 1).reshape(128, 6 * NJ)

        in_maps.append(
            {
                "planes": np.ascontiguousarray(planes),
                "tscal": np.ascontiguousarray(tsc.astype(np.float32)),
            }
        )
    return in_maps, perms, rsus


def _loss_pairs(pred_rows, tgt_rows):
    """Exact reference-form loss for pred_rows[...,4] vs tgt_rows[...,4] f32."""
    p, t = pred_rows, tgt_rows
    x1p = np.minimum(p[..., 0] + p[..., 2] / 2, np.float32(1.0))
    x0p = np.maximum(p[..., 0] - p[..., 2] / 2, np.float32(0.0))
    y1p = np.minimum(p[..., 1] + p[..., 3] / 2, np.float32(1.0))
    y0p = np.maximum(p[..., 1] - p[..., 3] / 2, np.float32(0.0))
    x1t = t[..., 0] + t[..., 2] / 2
    x0t = t[..., 0] - t[..., 2] / 2
    y1t = t[..., 1] + t[..., 3] / 2
    y0t = t[..., 1] - t[..., 3] / 2
    ox0 = np.maximum(x0t, x0p); ox1 = np.minimum(x1t, x1p)
    oy0 = np.maximum(y0t, y0p); oy1 = np.minimum(y1t, y1p)
    nov = (ox1 < ox0) | (oy1 < oy0)
    inter = (ox1 - ox0) * (oy1 - oy0)
    denom = p[..., 2] * p[..., 3] + t[..., 2] * t[..., 3] - inter + np.float32(EPS)
    iou = inter / denom
    d = p - t
    mse = np.sum(d * d, axis=-1) / np.float32(4.0)
    return np.where(nov, np.float32(1.0) + mse, np.float32(1.0) - iou)


def _host_greedy(vals, slots, perms, pred, tgt, rsumax=None, stats=None):
    """vals [T, NSH, 8] f32 desc slot-max bounds; slots [T, NSH, 8] slot ids."""
    # expand: slot s, mate m -> new_pos = s*AMB + m -> local orig via perm
    newpos = (
        slots[..., None] * AMB + np.arange(AMB)[None, None, None, :]
    )  # [T, NSH, 8, AMB]
    g = np.empty(newpos.shape, dtype=np.int64)
    for px in range(NP_SHARD):
        g[:, px] = perms[px][newpos[:, px]] + px * P_CORE
    g = g.reshape(T, -1)
    closs = _loss_pairs(pred[g], tgt[:, None, :]).astype(np.float64)

    order = np.lexsort((g, closs), axis=1)

    v8 = vals[:, :, 7].astype(np.float64)
    u_cap = np.min(v8, axis=1) * 1.03 + 1e-5
    if rsumax is not None:
        # fp16 plane quantization: |corner err| <= 2^-11 -> inter slack
        d = 2.0 ** -11
        u_cap = u_cap + (6 * d + 4 * d * d) * rsumax.astype(np.float64)
    u_cap = np.minimum(u_cap, 0.499999)
    bound = (1.0 - 2.0 * u_cap) / (1.0 - u_cap)

    taken = np.zeros(P_TOTAL, dtype=bool)
    sel = np.empty(T, dtype=np.int64)
    n_fb = 0
    for t in range(T):
        got = -1
        for d in order[t]:
            k = g[t, d]
            if not taken[k]:
                if closs[t, d] <= bound[t] - 1e-6:
                    got = k
                break
        if got < 0:
            n_fb += 1
            row = _loss_pairs(pred, tgt[t][None, :]).astype(np.float64)
            row[taken] = np.inf
            got = int(np.argmin(row))
        taken[got] = True
        sel[t] = got
    if stats is not None:
        stats["fallbacks"] = n_fb
    return np.float32(np.mean(_loss_pairs(pred[sel], tgt).astype(np.float64)))


def kernel(pred_bboxes, target_bboxes):
    from concourse.bass_utils import run_bass_kernel_spmd

    pred = np.asarray(pred_bboxes, dtype=np.float32)[0]
    tgt = np.asarray(target_bboxes, dtype=np.float32)[0]

    if "nc" not in _CACHE:
        _CACHE["nc"] = _build_nc()
    nc = _CACHE["nc"]

    in_maps, perms, rsus = _prep_core_inputs(pred, tgt)
    res = run_bass_kernel_spmd(nc, in_maps, list(range(N_CORES)))
    return _gather_and_reduce(res.results, perms, rsus, pred, tgt)


def _gather_and_reduce(results, perms, rsus, pred, tgt, stats=None):
    # m3[t, px, s]: device slot maxima; kq = m3 * rsu in f32 on host
    m3 = np.empty((T, NP_SHARD, NSLOT), np.float32)
    for c in range(N_CORES):
        px, ty = c % NP_SHARD, c // NP_SHARD
        tsl = slice(ty * T_CORE, (ty + 1) * T_CORE)
        o = results[c]["m3c"].astype(np.float32).reshape(128, NJ, NSLOT)
        m3[tsl, px] = o.transpose(1, 0, 2).reshape(T_CORE, NSLOT)

    kq = m3 * np.stack(rsus, axis=1)          # [T, NP_SHARD, NSLOT]
    part = np.argpartition(-kq, 8, axis=2)[:, :, :8]
    pv = np.take_along_axis(kq, part, axis=2)
    order8 = np.argsort(-pv, axis=2)
    slots = np.take_along_axis(part, order8, axis=2)     # [T, NSH, 8] desc
    vals = np.take_along_axis(pv, order8, axis=2)
    rsumax = np.stack(rsus, axis=1).max(axis=(1, 2))
    return _host_greedy(vals, slots, perms, pred, tgt, rsumax=rsumax, stats=stats)
